# revision 59
# baseline (speedup 1.0000x reference)
"""Trainium2 Bass kernel for nn_DeformableDynamicGather1D.

Sharding: 8 cores = 4 batches x 2 query-halves. Each core handles one batch's
feat and Q=4096 queries.

Host prep: feat is transposed to feat_T [L, C] and cast to bf16 on the host
(layout/precision prep, same class as the existing weight repacking). Router
weights are pre-cast to bf16; Wr has identity folded in; b3 is replicated to
[128, 12]; coords/cell are pre-cast to a bf16 [2, Q] tile for the MLP tail.

Device pipeline, per 1024-query chunk (4 chunks, software-pipelined so chunk
N's gathers run while chunk N-1 combines):

  1. Anchor: bilinear indices from coords (f32 DVE math, explicit [i0; i0+1]
     int32 offset pairs); indirect_dma_start row gather from feat_T (hardware
     dynamic DGE -- the DMA engine reads the offset table, Pool is not
     blocked); lerp on DVE (bf16); PE-transpose into channel-major rin.
  2. MLP on PE in bf16 (1 cycle/row): h = lrelu(rin@W1+b1) via ACT Prelu;
     g = lrelu(h@(Wr+I)+br); out3 = g@W3 + b3 (b3 added on DVE).
  3. Scalar stage: softplus via Abs/Exp/Ln, tanh/sigmoid via Exp + DVE
     reciprocal -- every ACT func lives in the natural_log_exp table, so no
     act-table reloads. Produces deform offset pairs and weights c0/c1.
  4. Deform: one indirect gather per chunk fetches all 5 taps' row-pairs
     (bf16); combine with scalar_tensor_tensor FMAs on DVE into a f32
     accumulator; per-chunk output DMA overlaps the next chunk's compute.

Query <-> tile coordinates: q = g*128 + p (tile [128 p, G g]).
"""
import os
import sys

for _p in ("/opt/trn_rl_repo", "/root/.axon_site/_ro/trn_rl_repo"):
    if os.path.isdir(_p) and _p not in sys.path:
        sys.path.append(_p)

import numpy as np
import concourse.bass as bass
import concourse.bacc as bacc
import concourse.tile as tile
from concourse import mybir
from concourse.bass import AP, IndirectOffsetOnAxis
from concourse.masks import make_identity

F32 = mybir.dt.float32
BF16 = mybir.dt.bfloat16
I32 = mybir.dt.int32
Act = mybir.ActivationFunctionType
Alu = mybir.AluOpType

P = 128          # partitions
G = 32           # q = g*128 + p
Q = P * G        # 4096 queries per core
C = 256          # channels
L = 4096         # feat length
H = 64           # hidden
K = 5            # taps
NCORES = 8
B, N = 4, 8192   # full problem
NI = int(os.environ.get("KERNEL_NI", "1024"))  # queries per chunk
NCH = Q // NI    # chunks
GPC = NI // P    # g-columns per chunk
GK = GPC * K

IXSCALE = np.float32(float(L - 1))          # 4095
DXSCALE = np.float32(2.0 / max(L - 1, 1))   # reference scale_x

# CoreSim lacks Prelu: sim-safe mode uses Identity + DVE leaky instead
# (numerically identical; only used by the local debug harness).
SIM_SAFE = os.environ.get("KERNEL_SIM_SAFE") == "1"

# Gather engine: per-tap blocking dma_gather with the wrapped-i16 idx layout
# (default), vs indirect_dma_start (hardware dynamic DGE -- passes CoreSim
# but aborts at runtime on this hardware stack; kept for reference).
INDIRECT = os.environ.get("KERNEL_GATHER", "dma_gather") == "indirect"


def _bc(ap2d: AP, extra: int) -> AP:
    """Broadcast a [p, n] AP to [p, n, extra] with stride-0 inner dim."""
    return AP(tensor=ap2d.tensor, offset=ap2d.offset,
              ap=[*ap2d.ap, [0, extra]])


def _bc_mid(ap2d: AP, mid: int) -> AP:
    """Broadcast a [p, n] AP to [p, mid, n] with stride-0 middle dim."""
    return AP(tensor=ap2d.tensor, offset=ap2d.offset,
              ap=[ap2d.ap[0], [0, mid], ap2d.ap[1]])


# Multi-queue SWDGE is unsupported under Tile: DMASW lane sems get locked to
# the first queue that uses them and the scheduler's lane rotation cannot be
# aligned with a per-call queue rotation (CoreSim flags the conflict).
NQUEUES = int(os.environ.get("KERNEL_NQ", "1"))

# Offload most combine FMAs to the Scalar engine: ACT computes the
# per-partition-scaled products (out = gathered_row * c), DVE reduces them
# with bf16 tensor_tensor adds (2x mode) into bf16 accumulators; output
# tensor becomes bf16 and the host converts to f32. g-columns >= ACT_GIS
# stay on the DVE scalar_tensor_tensor path.
ACT_OFFLOAD = os.environ.get("KERNEL_ACTOFF", "1") == "1"
ACT_GIS = 6


def build_program():
    nc = bacc.Bacc("TRN2", target_bir_lowering=False, debug=False,
                   num_devices=NCORES, num_swdge_queues=NQUEUES)

    featT = nc.dram_tensor("featT", [L, C], BF16, kind="ExternalInput")
    coords = nc.dram_tensor("coords", [Q], F32, kind="ExternalInput")
    xcb = nc.dram_tensor("xcb", [2, Q], BF16, kind="ExternalInput")
    w1a0 = nc.dram_tensor("w1a0", [128, H], BF16, kind="ExternalInput")
    w1a1 = nc.dram_tensor("w1a1", [128, H], BF16, kind="ExternalInput")
    wxc = nc.dram_tensor("wxc", [2, H], BF16, kind="ExternalInput")
    b1c = nc.dram_tensor("b1c", [H, 1], F32, kind="ExternalInput")
    wr1 = nc.dram_tensor("wr1", [H, H], BF16, kind="ExternalInput")
    brc = nc.dram_tensor("brc", [H, 1], F32, kind="ExternalInput")
    w3c = nc.dram_tensor("w3c", [H, 12], BF16, kind="ExternalInput")
    b3rep = nc.dram_tensor("b3rep", [P, 12], F32, kind="ExternalInput")
    base128 = nc.dram_tensor("base128", [P, K], F32, kind="ExternalInput")
    sel8 = nc.dram_tensor("sel8", [P, 8 * 128], F32, kind="ExternalInput")
    out = nc.dram_tensor("out", [Q, C], BF16 if ACT_OFFLOAD else F32,
                         kind="ExternalOutput")

    with tile.TileContext(nc) as tc:
        _body(nc, tc, featT, coords, xcb, w1a0, w1a1, wxc, b1c, wr1, brc,
              w3c, b3rep, base128, sel8, out)
    nc.compile()
    return nc


def _body(nc, tc, featT, coords, xcb, w1a0, w1a1, wxc, b1c, wr1, brc,
          w3c, b3rep, base128, sel8, out):
    import contextlib
    ctx = contextlib.ExitStack()
    with ctx:
        big = NI > 1024   # larger chunks: shrink buffer rings to fit SBUF
        const = ctx.enter_context(tc.tile_pool(name="const", bufs=1))
        rpool = ctx.enter_context(tc.tile_pool(name="rpool", bufs=1 if big else 2))
        gatha = ctx.enter_context(tc.tile_pool(name="gatha", bufs=1 if big else 2))
        gathd = ctx.enter_context(
            tc.tile_pool(name="gathd", bufs=2 if INDIRECT else (6 if big else 15)))
        fab = ctx.enter_context(tc.tile_pool(name="fab", bufs=1 if big else 2))
        sc = ctx.enter_context(tc.tile_pool(name="scal", bufs=2))
        wdp = ctx.enter_context(tc.tile_pool(name="wdp", bufs=2))
        accp = ctx.enter_context(tc.tile_pool(name="accp", bufs=1 if big else 2))
        prodp = ctx.enter_context(tc.tile_pool(name="prodp", bufs=12))
        tps = ctx.enter_context(tc.tile_pool(name="tps", bufs=2, space="PSUM"))
        mmps = ctx.enter_context(tc.tile_pool(name="mmps", bufs=2, space="PSUM"))
        l3ps = ctx.enter_context(tc.tile_pool(name="l3ps", bufs=2, space="PSUM"))

        ident = const.tile([P, P], BF16)
        make_identity(nc, ident[:])

        # weights / constants
        w1a0_sb = const.tile([128, H], BF16)
        w1a1_sb = const.tile([128, H], BF16)
        wxc_sb = const.tile([2, H], BF16)
        b1_sb = const.tile([H, 1], F32)
        wr1_sb = const.tile([H, H], BF16)
        br_sb = const.tile([H, 1], F32)
        w3_sb = const.tile([H, 12], BF16)
        b3_sb = const.tile([P, 12], F32)
        base_sb = const.tile([P, K], F32)
        xcb_sb = const.tile([2, Q], BF16)
        loads = [(w1a0_sb, w1a0), (w1a1_sb, w1a1), (wxc_sb, wxc),
                 (b1_sb, b1c), (wr1_sb, wr1), (br_sb, brc),
                 (w3_sb, w3c), (b3_sb, b3rep), (base_sb, base128),
                 (xcb_sb, xcb)]
        if not INDIRECT:
            sel_sb = const.tile([P, 8 * 128], F32)
            loads.append((sel_sb, sel8))
        for dst, src in loads:
            nc.sync.dma_start(out=dst[:], in_=src.ap())

        # feat_T row-pair view for dma_gather: idx i -> elems [256*i, +512)
        gsrc = AP(tensor=featT.ap().tensor, offset=0,
                  ap=[[C, L - 1], [1, 2 * C]])
        I16 = mybir.dt.int16
        qctr = [0]

        def next_q():
            q = qctr[0] % NQUEUES
            qctr[0] += 1
            return q

        def wrapped_idx(vf32_ap, nk, wrep):
            """Build replicated wrapped int16 idx tile from a query-major f32
            index tile V [128, nk*Gq] ((g, k)-major cols: n = g*nk + k) via 8
            selection matmuls; wrep is [128, nk, (128*Gq)//16] i16."""
            Gq = vf32_ap.shape[-1] // nk
            for a in range(8):
                psw = l3ps.tile([P, GK], F32, tag="psw", space="PSUM")
                nc.tensor.matmul(
                    out=psw[:, 0:nk * Gq], lhsT=sel_sb[:, a * 128:(a + 1) * 128],
                    rhs=vf32_ap, start=True, stop=True)
                dst = AP(tensor=wrep[:].tensor, offset=wrep[:].offset + a,
                         ap=[wrep[:].ap[0], [8 * Gq, nk], [8, Gq]])
                src = AP(tensor=psw[:].tensor, offset=psw[:].offset,
                         ap=[psw[:].ap[0], [1, nk], [nk, Gq]])
                nc.vector.tensor_copy(out=dst, in_=src)

        # ---- anchor index math, full Q upfront (query-major [P, G]) ----
        xq = const.tile([P, G], F32)
        nc.sync.dma_start(
            out=xq[:],
            in_=AP(tensor=coords.ap().tensor, offset=0, ap=[[1, P], [P, G]]))
        ixf = const.tile([P, G], F32)
        nc.vector.tensor_scalar(out=ixf[:], in0=xq[:], scalar1=1.0,
                                scalar2=0.5, op0=Alu.add, op1=Alu.mult)
        nc.vector.tensor_scalar(out=ixf[:], in0=ixf[:], scalar1=float(IXSCALE),
                                scalar2=0.0, op0=Alu.mult, op1=Alu.max)
        nc.vector.tensor_scalar(out=ixf[:], in0=ixf[:], scalar1=float(IXSCALE),
                                scalar2=None, op0=Alu.min)
        # i0 = min(floor(ix), L-2); frac = ix - i0 (floor via int convert +
        # fixup, correct for both trunc and round-nearest convert modes)
        fraca = const.tile([P, G], F32)
        i0fa = const.tile([P, G], F32)
        ti_a = const.tile([P, G], I32)
        nc.vector.tensor_copy(out=ti_a[:], in_=ixf[:])
        nc.vector.tensor_copy(out=i0fa[:], in_=ti_a[:])
        gt_a = const.tile([P, G], F32)
        nc.vector.tensor_tensor(out=gt_a[:], in0=i0fa[:], in1=ixf[:],
                                op=Alu.is_gt)
        nc.vector.tensor_tensor(out=i0fa[:], in0=i0fa[:], in1=gt_a[:],
                                op=Alu.subtract)
        nc.vector.tensor_scalar(out=i0fa[:], in0=i0fa[:], scalar1=float(L - 2),
                                scalar2=None, op0=Alu.min)
        nc.vector.tensor_tensor(out=fraca[:], in0=ixf[:], in1=i0fa[:],
                                op=Alu.subtract)
        if INDIRECT:
            # anchor offset pairs [P, G, 2] i32 = [i0; i0+1]
            aidx = const.tile([P, G, 2], I32)
            i0p1a = const.tile([P, G], F32)
            nc.vector.tensor_scalar(out=i0p1a[:], in0=i0fa[:], scalar1=1.0,
                                    scalar2=None, op0=Alu.add)
            nc.vector.tensor_copy(out=aidx[:, :, 0], in_=i0fa[:])
            nc.vector.tensor_copy(out=aidx[:, :, 1], in_=i0p1a[:])
        else:
            wrapA = const.tile([P, 1, Q // 16], I16)
            wrapped_idx(i0fa[:], 1, wrapA)

        # ---------------- software-pipelined chunk loop ----------------
        # fe(ch) (which issues ch's gathers) is emitted before combine(ch-1)
        # so gathers stay a chunk ahead of the combine consumers.
        pend = [None]

        def front_end(ch):
            g0 = ch * GPC

            # anchor row-pair gather: out[p, gi, r, :] = featT[i0(+r), :]
            Ga = gatha.tile([P, GPC, 2 * C], BF16, tag="ga")
            if INDIRECT:
                nc.gpsimd.indirect_dma_start(
                    out=Ga[:].rearrange("p g (r c) -> p (g r) c", r=2),
                    out_offset=None,
                    in_=featT.ap(),
                    in_offset=IndirectOffsetOnAxis(
                        ap=aidx[:, g0:g0 + GPC, :], axis=0))
            else:
                for s in range(NI // 1024):
                    f0 = ch * (NI // 16) + s * 64
                    nc.gpsimd.dma_gather(
                        out_ap=Ga[:, s * 8:(s + 1) * 8, :], in_ap=gsrc,
                        idxs_ap=wrapA[:, 0, f0:f0 + 64],
                        num_idxs=1024, num_idxs_reg=1024, elem_size=2 * C,
                        elem_step=C, queue_num=next_q())

            # lerp: d = f1 - f0 (bf16 2x); fa = frac*d + f0 per g-column
            d = fab.tile([P, GPC, C], BF16, tag="dl")
            nc.vector.tensor_tensor(out=d[:], in0=Ga[:, :, C:2 * C],
                                    in1=Ga[:, :, 0:C], op=Alu.subtract)
            rin0 = rpool.tile([P, NI], BF16, tag="rin0")
            rin1 = rpool.tile([P, NI], BF16, tag="rin1")
            for gi in range(GPC):
                g = g0 + gi
                fa = fab.tile([P, C], BF16, tag="fa")
                nc.vector.scalar_tensor_tensor(
                    out=fa[:], in0=d[:, gi, :], scalar=fraca[:, g:g + 1],
                    in1=Ga[:, gi, 0:C], op0=Alu.mult, op1=Alu.add)
                for hh in range(2):
                    tp = tps.tile([P, P], BF16, tag="tp", space="PSUM")
                    nc.tensor.transpose(out=tp[:],
                                        in_=fa[:, hh * 128:(hh + 1) * 128],
                                        identity=ident[:])
                    rdst = (rin0 if hh == 0 else rin1)
                    nc.scalar.copy(out=rdst[:, gi * 128:(gi + 1) * 128],
                                   in_=tp[:])

            # MLP (bf16): h = lrelu(rin@W1 + b1); g = lrelu(h + h@Wr + br)
            hb = rpool.tile([H, NI], BF16, tag="hb")
            gg = rpool.tile([H, NI], BF16, tag="gg")
            for n in range(NI // 512):
                sl = slice(n * 512, (n + 1) * 512)
                gsl = slice(ch * NI + n * 512, ch * NI + (n + 1) * 512)
                ps1 = mmps.tile([H, 512], F32, tag="ps1", space="PSUM")
                nc.tensor.matmul(out=ps1[:], lhsT=w1a0_sb[:], rhs=rin0[:, sl],
                                 start=True, stop=False)
                nc.tensor.matmul(out=ps1[:], lhsT=w1a1_sb[:], rhs=rin1[:, sl],
                                 start=False, stop=False)
                nc.tensor.matmul(out=ps1[:], lhsT=wxc_sb[:], rhs=xcb_sb[:, gsl],
                                 start=False, stop=True)
                def lrelu(dst, ps, bias_sb, tag):
                    if not SIM_SAFE:
                        nc.scalar.activation(out=dst, in_=ps, func=Act.Prelu,
                                             bias=bias_sb[:, :], scale=1.0,
                                             alpha=0.2)
                    else:
                        t = sc.tile([H, 512], F32, tag=tag)
                        nc.scalar.activation(out=t[:], in_=ps,
                                             func=Act.Identity,
                                             bias=bias_sb[:, :], scale=1.0)
                        nc.vector.scalar_tensor_tensor(
                            out=dst, in0=t[:], scalar=0.2, in1=t[:],
                            op0=Alu.mult, op1=Alu.max)

                lrelu(hb[:, sl], ps1[:], b1_sb, "lr1")
                ps2 = mmps.tile([H, 512], F32, tag="ps1", space="PSUM")
                nc.tensor.matmul(out=ps2[:], lhsT=wr1_sb[:], rhs=hb[:, sl],
                                 start=True, stop=True)
                lrelu(gg[:, sl], ps2[:], br_sb, "lr2")

            # out3 = g@W3 (+ b3 on DVE), query-major [P, GPC, 12]
            o3 = rpool.tile([P, GPC, 12], F32, tag="o3")
            for gi in range(GPC):
                ps3 = l3ps.tile([P, 12], F32, tag="ps3", space="PSUM")
                nc.tensor.matmul(out=ps3[:],
                                 lhsT=gg[:, gi * 128:(gi + 1) * 128],
                                 rhs=w3_sb[:], start=True, stop=True)
                nc.scalar.copy(out=o3[:, gi, :], in_=ps3[:])
            nc.vector.tensor_tensor(out=o3[:], in0=o3[:],
                                    in1=_bc_mid(b3_sb[:], GPC), op=Alu.add)

            # ---- scalar stage (tiles [P, GPC] / [P, GK] f32) ----
            def softplus(dst, src_ap, tag):
                a = sc.tile([P, GPC], F32, tag=tag + "a")
                nc.scalar.activation(out=a[:], in_=src_ap, func=Act.Abs)
                e = sc.tile([P, GPC], F32, tag=tag + "e")
                nc.scalar.activation(out=e[:], in_=a[:], func=Act.Exp,
                                     scale=-1.0)
                lg = sc.tile([P, GPC], F32, tag=tag + "l")
                nc.scalar.activation(out=lg[:], in_=e[:], func=Act.Ln,
                                     bias=1.0, scale=1.0)
                m = sc.tile([P, GPC], F32, tag=tag + "m")
                nc.vector.tensor_scalar(out=m[:], in0=src_ap, scalar1=0.0,
                                        scalar2=None, op0=Alu.max)
                nc.vector.tensor_tensor(out=dst, in0=lg[:], in1=m[:],
                                        op=Alu.add)

            r_t = sc.tile([P, GPC], F32, tag="rt")
            softplus(r_t[:], o3[:, :, 0], "spr")
            nc.vector.tensor_scalar(out=r_t[:], in0=r_t[:], scalar1=0.3,
                                    scalar2=2.0, op0=Alu.add, op1=Alu.min)
            sg_t = sc.tile([P, GPC], F32, tag="sgt")
            softplus(sg_t[:], o3[:, :, 1], "sps")
            nc.vector.tensor_scalar(out=sg_t[:], in0=sg_t[:], scalar1=0.5,
                                    scalar2=3.0, op0=Alu.add, op1=Alu.min)
            s2 = sc.tile([P, GPC], F32, tag="s2")
            nc.vector.tensor_tensor(out=s2[:], in0=sg_t[:], in1=sg_t[:],
                                    op=Alu.mult)
            nc.vector.tensor_scalar(out=s2[:], in0=s2[:], scalar1=4.0,
                                    scalar2=1e-8, op0=Alu.mult, op1=Alu.add)
            rs = sc.tile([P, GPC], F32, tag="rs")
            nc.vector.reciprocal(out=rs[:], in_=s2[:])

            # tanh(res_raw) via exp: 1 - 2/(exp(2x)+1)
            th = sc.tile([P, GK], F32, tag="th")
            nc.scalar.activation(out=th[:], in_=o3[:, :, 2:7], func=Act.Exp,
                                 scale=2.0)
            nc.vector.tensor_scalar(out=th[:], in0=th[:], scalar1=1.0,
                                    scalar2=None, op0=Alu.add)
            nc.vector.reciprocal(out=th[:], in_=th[:])
            nc.vector.tensor_scalar(out=th[:], in0=th[:], scalar1=-2.0,
                                    scalar2=1.0, op0=Alu.mult, op1=Alu.add)
            # sigmoid(gate_raw) via exp: 1/(exp(-x)+1)
            gt = sc.tile([P, GK], F32, tag="gt")
            nc.scalar.activation(out=gt[:], in_=o3[:, :, 7:12], func=Act.Exp,
                                 scale=-1.0)
            nc.vector.tensor_scalar(out=gt[:], in0=gt[:], scalar1=1.0,
                                    scalar2=None, op0=Alu.add)
            nc.vector.reciprocal(out=gt[:], in_=gt[:])

            off = sc.tile([P, GK], F32, tag="off")
            nc.vector.tensor_tensor(out=off[:], in0=_bc(r_t[:], K),
                                    in1=_bc_mid(base_sb[:], GPC), op=Alu.mult)
            nc.vector.scalar_tensor_tensor(out=off[:], in0=th[:], scalar=0.5,
                                           in1=off[:], op0=Alu.mult,
                                           op1=Alu.add)
            dix = sc.tile([P, GK], F32, tag="dix")
            nc.vector.scalar_tensor_tensor(
                out=dix[:], in0=off[:], scalar=float(DXSCALE),
                in1=_bc(xq[:, g0:g0 + GPC], K), op0=Alu.mult, op1=Alu.add)
            nc.vector.tensor_scalar(out=dix[:], in0=dix[:], scalar1=1.0,
                                    scalar2=0.5, op0=Alu.add, op1=Alu.mult)
            nc.vector.tensor_scalar(out=dix[:], in0=dix[:],
                                    scalar1=float(IXSCALE), scalar2=0.0,
                                    op0=Alu.mult, op1=Alu.max)
            nc.vector.tensor_scalar(out=dix[:], in0=dix[:],
                                    scalar1=float(IXSCALE), scalar2=None,
                                    op0=Alu.min)
            fracd = sc.tile([P, GK], F32, tag="fracd")
            i0fd = sc.tile([P, GK], F32, tag="i0fd")
            ti_d = sc.tile([P, GK], I32, tag="tid")
            nc.vector.tensor_copy(out=ti_d[:], in_=dix[:])
            nc.vector.tensor_copy(out=i0fd[:], in_=ti_d[:])
            gt_d = sc.tile([P, GK], F32, tag="gtd")
            nc.vector.tensor_tensor(out=gt_d[:], in0=i0fd[:], in1=dix[:],
                                    op=Alu.is_gt)
            nc.vector.tensor_tensor(out=i0fd[:], in0=i0fd[:], in1=gt_d[:],
                                    op=Alu.subtract)
            nc.vector.tensor_scalar(out=i0fd[:], in0=i0fd[:],
                                    scalar1=float(L - 2), scalar2=None,
                                    op0=Alu.min)
            nc.vector.tensor_tensor(out=fracd[:], in0=dix[:], in1=i0fd[:],
                                    op=Alu.subtract)

            o2 = sc.tile([P, GK], F32, tag="o2")
            nc.vector.tensor_tensor(out=o2[:], in0=off[:], in1=off[:],
                                    op=Alu.mult)
            nc.vector.tensor_tensor(out=o2[:], in0=o2[:], in1=_bc(rs[:], K),
                                    op=Alu.mult)
            w_t = sc.tile([P, GK], F32, tag="wt")
            nc.scalar.activation(out=w_t[:], in_=o2[:], func=Act.Exp,
                                 scale=-0.5)
            nc.vector.tensor_tensor(out=w_t[:], in0=w_t[:], in1=gt[:],
                                    op=Alu.mult)
            wsum = sc.tile([P, GPC], F32, tag="wsum")
            w_v = w_t[:].rearrange("p (g k) -> p g k", k=K)
            nc.vector.tensor_reduce(out=wsum[:], in_=w_v,
                                    axis=mybir.AxisListType.X, op=Alu.add)
            nc.vector.tensor_scalar(out=wsum[:], in0=wsum[:], scalar1=1e-8,
                                    scalar2=None, op0=Alu.add)
            rn = sc.tile([P, GPC], F32, tag="rn")
            nc.vector.reciprocal(out=rn[:], in_=wsum[:])
            wn = sc.tile([P, GK], F32, tag="wn")
            nc.vector.tensor_tensor(out=wn[:], in0=w_t[:], in1=_bc(rn[:], K),
                                    op=Alu.mult)
            c1 = sc.tile([P, GK], F32, tag="c1")
            nc.vector.tensor_tensor(out=c1[:], in0=wn[:], in1=fracd[:],
                                    op=Alu.mult)
            c0 = sc.tile([P, GK], F32, tag="c0")
            nc.vector.tensor_tensor(out=c0[:], in0=wn[:], in1=c1[:],
                                    op=Alu.subtract)

            if INDIRECT:
                # deform offset pairs [P, GK, 2] i32 = [i0; i0+1]
                didx = sc.tile([P, GK, 2], I32, tag="didx")
                i0p1 = sc.tile([P, GK], F32, tag="i0p1")
                nc.vector.tensor_scalar(out=i0p1[:], in0=i0fd[:], scalar1=1.0,
                                        scalar2=None, op0=Alu.add)
                nc.vector.tensor_copy(out=didx[:, :, 0], in_=i0fd[:])
                nc.vector.tensor_copy(out=didx[:, :, 1], in_=i0p1[:])

                # one indirect gather for all 5 taps' row-pairs of this chunk
                Gd = gathd.tile([P, GK, 2 * C], BF16, tag="gd")
                nc.gpsimd.indirect_dma_start(
                    out=Gd[:].rearrange("p n (r c) -> p (n r) c", r=2),
                    out_offset=None,
                    in_=featT.ap(),
                    in_offset=IndirectOffsetOnAxis(ap=didx[:], axis=0))

                def gsl(k, gi, half):
                    n = gi * K + k
                    return Gd[:, n, half * C:(half + 1) * C]
            else:
                wrepD = wdp.tile([P, K, NI // 16], I16, tag="wd")
                wrapped_idx(i0fd[:], K, wrepD)
                Gds = []
                for k in range(K):
                    Gd = gathd.tile([P, GPC, 2 * C], BF16, tag="gd")
                    nc.gpsimd.dma_gather(
                        out_ap=Gd[:], in_ap=gsrc,
                        idxs_ap=wrepD[:, k, :], num_idxs=NI, num_idxs_reg=NI,
                        elem_size=2 * C, elem_step=C, queue_num=next_q())
                    Gds.append(Gd)

                def gsl(k, gi, half):
                    return Gds[k][:, gi, half * C:(half + 1) * C]
            return gsl, c0, c1

        def combine_and_store(gsl, c0, c1, ch):
            accV = accp.tile([P, GPC, C], BF16 if ACT_OFFLOAD else F32,
                             tag="accV")
            for gi in range(GPC):
                acc = accV[:, gi, :]
                if ACT_OFFLOAD and gi < ACT_GIS:
                    # ACT: prod = row * c (per-partition scale); DVE: bf16
                    # tensor_tensor adds (2x mode) reduce the 10 products.
                    prods = []
                    for k in range(K):
                        n = gi * K + k
                        for half, cw in ((0, c0), (1, c1)):
                            pr = prodp.tile([P, C], BF16, tag="pr")
                            nc.scalar.activation(
                                out=pr[:], in_=gsl(k, gi, half), func=Act.Copy,
                                scale=cw[:, n:n + 1])
                            prods.append(pr)
                    nc.vector.tensor_tensor(out=acc, in0=prods[0][:],
                                            in1=prods[1][:], op=Alu.add)
                    for pr in prods[2:]:
                        nc.vector.tensor_tensor(out=acc, in0=acc, in1=pr[:],
                                                op=Alu.add)
                    continue
                for k in range(K):
                    n = gi * K + k
                    csc0 = c0[:, n:n + 1]
                    csc1 = c1[:, n:n + 1]
                    if k == 0:
                        nc.vector.tensor_scalar(out=acc, in0=gsl(k, gi, 0),
                                                scalar1=csc0, scalar2=None,
                                                op0=Alu.mult)
                    else:
                        nc.vector.scalar_tensor_tensor(
                            out=acc, in0=gsl(k, gi, 0), scalar=csc0, in1=acc,
                            op0=Alu.mult, op1=Alu.add)
                    nc.vector.scalar_tensor_tensor(
                        out=acc, in0=gsl(k, gi, 1), scalar=csc1, in1=acc,
                        op0=Alu.mult, op1=Alu.add)
            outv = out.ap().rearrange("(g p) c -> p g c", p=P)
            g0 = ch * GPC
            nc.sync.dma_start(out=outv[:, g0:g0 + GPC, :], in_=accV[:])

        for ch in range(NCH):
            args = front_end(ch)
            if pend[0] is not None:
                combine_and_store(*pend[0])
            pend[0] = (*args, ch)
        combine_and_store(*pend[0])


_PROGRAM = None


def _get_program():
    global _PROGRAM
    if _PROGRAM is None:
        _PROGRAM = build_program()
    return _PROGRAM


def make_in_maps(feat_1d, coords_1d, cell_1d, W1, b1, Wr, br, W3, b3):
    """Build the 8 per-core input dicts from full inputs."""
    from ml_dtypes import bfloat16
    f32 = np.float32
    W1 = np.asarray(W1, f32)
    wr1 = (np.asarray(Wr, f32) + np.eye(H, dtype=f32)).astype(bfloat16)
    base = np.array([-2.0, -1.0, 0.0, 1.0, 2.0], f32)
    base128 = np.broadcast_to(base, (P, K)).copy()
    b3rep = np.broadcast_to(np.asarray(b3, f32), (P, 12)).copy()
    sel = np.zeros((P, 8, 128), f32)
    for a in range(8):
        for m in range(128):
            sel[16 * a + m % 16, a, m] = 1.0
    shared = {
        "sel8": sel.reshape(P, 8 * 128),
        "w1a0": np.ascontiguousarray(W1[0:128]).astype(bfloat16),
        "w1a1": np.ascontiguousarray(W1[128:256]).astype(bfloat16),
        "wxc": np.ascontiguousarray(W1[256:258]).astype(bfloat16),
        "b1c": np.asarray(b1, f32).reshape(H, 1).copy(),
        "wr1": wr1,
        "brc": np.asarray(br, f32).reshape(H, 1).copy(),
        "w3c": np.asarray(W3, f32).astype(bfloat16),
        "b3rep": b3rep,
        "base128": base128,
    }
    featTs = [np.ascontiguousarray(np.asarray(feat_1d[b], f32).T)
              .astype(bfloat16) for b in range(B)]
    in_maps = []
    for core in range(NCORES):
        b = core // 2
        s = core % 2
        sl = slice(s * Q, (s + 1) * Q)
        cds = np.ascontiguousarray(np.asarray(coords_1d[b, sl, 0], f32))
        cel = np.ascontiguousarray(np.asarray(cell_1d[b, sl, 0], f32))
        xcb = np.stack([cds, cel]).astype(bfloat16)
        in_maps.append({
            "featT": featTs[b],
            "coords": cds,
            "xcb": xcb,
            **shared,
        })
    return in_maps


def kernel(feat_1d, coords_1d, cell_1d, W1, b1, Wr, br, W3, b3):
    from concourse.bass_utils import run_bass_kernel_spmd
    nc = _get_program()
    in_maps = make_in_maps(feat_1d, coords_1d, cell_1d, W1, b1, Wr, br, W3, b3)
    res = run_bass_kernel_spmd(nc, in_maps, core_ids=list(range(NCORES)))
    outf = np.zeros((B, N, C), np.float32)
    for core in range(NCORES):
        b = core // 2
        s = core % 2
        outf[b, s * Q:(s + 1) * Q, :] = np.asarray(
            res.results[core]["out"]).astype(np.float32)
    return outf


# revision 62
# speedup vs baseline: 1.0868x; 1.0868x over previous
"""Trainium2 Bass kernel for nn_DeformableDynamicGather1D.

Sharding: 8 cores = 4 batches x 2 query-halves. Each core handles one batch's
feat and Q=4096 queries.

Host prep: feat is transposed to feat_T [L, C] and cast to bf16 on the host
(layout/precision prep, same class as the existing weight repacking). Router
weights are pre-cast to bf16; Wr has identity folded in; b3 is replicated to
[128, 12]; coords/cell are pre-cast to a bf16 [2, Q] tile for the MLP tail.

Device pipeline, per 1024-query chunk (4 chunks, software-pipelined so chunk
N's gathers run while chunk N-1 combines):

  1. Anchor: bilinear indices from coords (f32 DVE math, explicit [i0; i0+1]
     int32 offset pairs); indirect_dma_start row gather from feat_T (hardware
     dynamic DGE -- the DMA engine reads the offset table, Pool is not
     blocked); lerp on DVE (bf16); PE-transpose into channel-major rin.
  2. MLP on PE in bf16 (1 cycle/row): h = lrelu(rin@W1+b1) via ACT Prelu;
     g = lrelu(h@(Wr+I)+br); out3 = g@W3 + b3 (b3 added on DVE).
  3. Scalar stage: softplus via Abs/Exp/Ln, tanh/sigmoid via Exp + DVE
     reciprocal -- every ACT func lives in the natural_log_exp table, so no
     act-table reloads. Produces deform offset pairs and weights c0/c1.
  4. Deform: one indirect gather per chunk fetches all 5 taps' row-pairs
     (bf16); combine with scalar_tensor_tensor FMAs on DVE into a f32
     accumulator; per-chunk output DMA overlaps the next chunk's compute.

Query <-> tile coordinates: q = g*128 + p (tile [128 p, G g]).
"""
import os
import sys

for _p in ("/opt/trn_rl_repo", "/root/.axon_site/_ro/trn_rl_repo"):
    if os.path.isdir(_p) and _p not in sys.path:
        sys.path.append(_p)

import numpy as np
import concourse.bass as bass
import concourse.bacc as bacc
import concourse.tile as tile
from concourse import mybir
from concourse.bass import AP, IndirectOffsetOnAxis
from concourse.masks import make_identity

F32 = mybir.dt.float32
BF16 = mybir.dt.bfloat16
I32 = mybir.dt.int32
Act = mybir.ActivationFunctionType
Alu = mybir.AluOpType

P = 128          # partitions
G = 32           # q = g*128 + p
Q = P * G        # 4096 queries per core
C = 256          # channels
L = 4096         # feat length
H = 64           # hidden
K = 5            # taps
NCORES = 8
B, N = 4, 8192   # full problem
NI = int(os.environ.get("KERNEL_NI", "1024"))  # queries per chunk
NCH = Q // NI    # chunks
GPC = NI // P    # g-columns per chunk
GK = GPC * K

IXSCALE = np.float32(float(L - 1))          # 4095
DXSCALE = np.float32(2.0 / max(L - 1, 1))   # reference scale_x

# CoreSim lacks Prelu: sim-safe mode uses Identity + DVE leaky instead
# (numerically identical; only used by the local debug harness).
SIM_SAFE = os.environ.get("KERNEL_SIM_SAFE") == "1"

# Gather engine: per-tap blocking dma_gather with the wrapped-i16 idx layout
# (default), vs indirect_dma_start (hardware dynamic DGE -- passes CoreSim
# but aborts at runtime on this hardware stack; kept for reference).
INDIRECT = os.environ.get("KERNEL_GATHER", "dma_gather") == "indirect"


def _bc(ap2d: AP, extra: int) -> AP:
    """Broadcast a [p, n] AP to [p, n, extra] with stride-0 inner dim."""
    return AP(tensor=ap2d.tensor, offset=ap2d.offset,
              ap=[*ap2d.ap, [0, extra]])


def _bc_mid(ap2d: AP, mid: int) -> AP:
    """Broadcast a [p, n] AP to [p, mid, n] with stride-0 middle dim."""
    return AP(tensor=ap2d.tensor, offset=ap2d.offset,
              ap=[ap2d.ap[0], [0, mid], ap2d.ap[1]])


# Multi-queue SWDGE is unsupported under Tile: DMASW lane sems get locked to
# the first queue that uses them and the scheduler's lane rotation cannot be
# aligned with a per-call queue rotation (CoreSim flags the conflict).
NQUEUES = int(os.environ.get("KERNEL_NQ", "1"))

# Offload most combine FMAs to the Scalar engine: ACT computes the
# per-partition-scaled products (out = gathered_row * c), DVE reduces them
# with bf16 tensor_tensor adds (2x mode) into bf16 accumulators; output
# tensor becomes bf16 and the host converts to f32. g-columns >= ACT_GIS
# stay on the DVE scalar_tensor_tensor path.
ACT_OFFLOAD = os.environ.get("KERNEL_ACTOFF", "1") == "1"
ACT_GIS = 6


def build_program():
    nc = bacc.Bacc("TRN2", target_bir_lowering=False, debug=False,
                   num_devices=NCORES, num_swdge_queues=NQUEUES)

    featT = nc.dram_tensor("featT", [L, C], BF16, kind="ExternalInput")
    coords = nc.dram_tensor("coords", [Q], F32, kind="ExternalInput")
    xcb = nc.dram_tensor("xcb", [2, Q], BF16, kind="ExternalInput")
    w1a0 = nc.dram_tensor("w1a0", [128, H], BF16, kind="ExternalInput")
    w1a1 = nc.dram_tensor("w1a1", [128, H], BF16, kind="ExternalInput")
    wxc = nc.dram_tensor("wxc", [2, H], BF16, kind="ExternalInput")
    b1c = nc.dram_tensor("b1c", [H, 1], F32, kind="ExternalInput")
    wr1 = nc.dram_tensor("wr1", [H, H], BF16, kind="ExternalInput")
    brc = nc.dram_tensor("brc", [H, 1], F32, kind="ExternalInput")
    w3c = nc.dram_tensor("w3c", [H, 12], BF16, kind="ExternalInput")
    b3rep = nc.dram_tensor("b3rep", [P, 12], F32, kind="ExternalInput")
    base128 = nc.dram_tensor("base128", [P, K], F32, kind="ExternalInput")
    sel8 = nc.dram_tensor("sel8", [P, 8 * 128], F32, kind="ExternalInput")
    out = nc.dram_tensor("out", [Q, C], BF16 if ACT_OFFLOAD else F32,
                         kind="ExternalOutput")

    with tile.TileContext(nc) as tc:
        _body(nc, tc, featT, coords, xcb, w1a0, w1a1, wxc, b1c, wr1, brc,
              w3c, b3rep, base128, sel8, out)
    nc.compile()
    return nc


def _body(nc, tc, featT, coords, xcb, w1a0, w1a1, wxc, b1c, wr1, brc,
          w3c, b3rep, base128, sel8, out):
    import contextlib
    ctx = contextlib.ExitStack()
    with ctx:
        big = NI > 1024   # larger chunks: shrink buffer rings to fit SBUF
        const = ctx.enter_context(tc.tile_pool(name="const", bufs=1))
        rpool = ctx.enter_context(tc.tile_pool(name="rpool", bufs=1 if big else 2))
        gatha = ctx.enter_context(tc.tile_pool(name="gatha", bufs=1 if big else 4))
        gathd = ctx.enter_context(
            tc.tile_pool(name="gathd", bufs=2 if INDIRECT else (6 if big else 10)))
        fab = ctx.enter_context(tc.tile_pool(name="fab", bufs=1 if big else 2))
        sc = ctx.enter_context(tc.tile_pool(name="scal", bufs=2))
        wdp = ctx.enter_context(tc.tile_pool(name="wdp", bufs=2))
        accp = ctx.enter_context(tc.tile_pool(name="accp", bufs=1 if big else 2))
        prodp = ctx.enter_context(tc.tile_pool(name="prodp", bufs=12))
        tps = ctx.enter_context(tc.tile_pool(name="tps", bufs=2, space="PSUM"))
        mmps = ctx.enter_context(tc.tile_pool(name="mmps", bufs=2, space="PSUM"))
        l3ps = ctx.enter_context(tc.tile_pool(name="l3ps", bufs=2, space="PSUM"))

        ident = const.tile([P, P], BF16)
        make_identity(nc, ident[:])

        # weights / constants
        w1a0_sb = const.tile([128, H], BF16)
        w1a1_sb = const.tile([128, H], BF16)
        wxc_sb = const.tile([2, H], BF16)
        b1_sb = const.tile([H, 1], F32)
        wr1_sb = const.tile([H, H], BF16)
        br_sb = const.tile([H, 1], F32)
        w3_sb = const.tile([H, 12], BF16)
        b3_sb = const.tile([P, 12], F32)
        base_sb = const.tile([P, K], F32)
        xcb_sb = const.tile([2, Q], BF16)
        loads = [(w1a0_sb, w1a0), (w1a1_sb, w1a1), (wxc_sb, wxc),
                 (b1_sb, b1c), (wr1_sb, wr1), (br_sb, brc),
                 (w3_sb, w3c), (b3_sb, b3rep), (base_sb, base128),
                 (xcb_sb, xcb)]
        if not INDIRECT:
            sel_sb = const.tile([P, 8 * 128], F32)
            loads.append((sel_sb, sel8))
        for dst, src in loads:
            nc.sync.dma_start(out=dst[:], in_=src.ap())

        # feat_T row-pair view for dma_gather: idx i -> elems [256*i, +512)
        gsrc = AP(tensor=featT.ap().tensor, offset=0,
                  ap=[[C, L - 1], [1, 2 * C]])
        I16 = mybir.dt.int16
        qctr = [0]

        def next_q():
            q = qctr[0] % NQUEUES
            qctr[0] += 1
            return q

        def wrapped_idx(vf32_ap, nk, wrep):
            """Build replicated wrapped int16 idx tile from a query-major f32
            index tile V [128, nk*Gq] ((g, k)-major cols: n = g*nk + k) via 8
            selection matmuls; wrep is [128, nk, (128*Gq)//16] i16."""
            Gq = vf32_ap.shape[-1] // nk
            for a in range(8):
                psw = l3ps.tile([P, GK], F32, tag="psw", space="PSUM")
                nc.tensor.matmul(
                    out=psw[:, 0:nk * Gq], lhsT=sel_sb[:, a * 128:(a + 1) * 128],
                    rhs=vf32_ap, start=True, stop=True)
                dst = AP(tensor=wrep[:].tensor, offset=wrep[:].offset + a,
                         ap=[wrep[:].ap[0], [8 * Gq, nk], [8, Gq]])
                src = AP(tensor=psw[:].tensor, offset=psw[:].offset,
                         ap=[psw[:].ap[0], [1, nk], [nk, Gq]])
                nc.vector.tensor_copy(out=dst, in_=src)

        # ---- anchor index math, full Q upfront (query-major [P, G]) ----
        xq = const.tile([P, G], F32)
        nc.sync.dma_start(
            out=xq[:],
            in_=AP(tensor=coords.ap().tensor, offset=0, ap=[[1, P], [P, G]]))
        ixf = const.tile([P, G], F32)
        nc.vector.tensor_scalar(out=ixf[:], in0=xq[:], scalar1=1.0,
                                scalar2=0.5, op0=Alu.add, op1=Alu.mult)
        nc.vector.tensor_scalar(out=ixf[:], in0=ixf[:], scalar1=float(IXSCALE),
                                scalar2=0.0, op0=Alu.mult, op1=Alu.max)
        nc.vector.tensor_scalar(out=ixf[:], in0=ixf[:], scalar1=float(IXSCALE),
                                scalar2=None, op0=Alu.min)
        # i0 = min(floor(ix), L-2); frac = ix - i0 (floor via int convert +
        # fixup, correct for both trunc and round-nearest convert modes)
        fraca = const.tile([P, G], F32)
        i0fa = const.tile([P, G], F32)
        ti_a = const.tile([P, G], I32)
        nc.vector.tensor_copy(out=ti_a[:], in_=ixf[:])
        nc.vector.tensor_copy(out=i0fa[:], in_=ti_a[:])
        gt_a = const.tile([P, G], F32)
        nc.vector.tensor_tensor(out=gt_a[:], in0=i0fa[:], in1=ixf[:],
                                op=Alu.is_gt)
        nc.vector.tensor_tensor(out=i0fa[:], in0=i0fa[:], in1=gt_a[:],
                                op=Alu.subtract)
        nc.vector.tensor_scalar(out=i0fa[:], in0=i0fa[:], scalar1=float(L - 2),
                                scalar2=None, op0=Alu.min)
        nc.vector.tensor_tensor(out=fraca[:], in0=ixf[:], in1=i0fa[:],
                                op=Alu.subtract)
        if INDIRECT:
            # anchor offset pairs [P, G, 2] i32 = [i0; i0+1]
            aidx = const.tile([P, G, 2], I32)
            i0p1a = const.tile([P, G], F32)
            nc.vector.tensor_scalar(out=i0p1a[:], in0=i0fa[:], scalar1=1.0,
                                    scalar2=None, op0=Alu.add)
            nc.vector.tensor_copy(out=aidx[:, :, 0], in_=i0fa[:])
            nc.vector.tensor_copy(out=aidx[:, :, 1], in_=i0p1a[:])
        else:
            wrapA = const.tile([P, 1, Q // 16], I16)
            wrapped_idx(i0fa[:], 1, wrapA)

        # All anchor gathers issued upfront: each chunk's MLP/scalar chain
        # (which gates its deform gathers) then overlaps the previous chunk's
        # deform burst instead of queueing its anchor behind it on Pool.
        Gas = []
        for ch in range(NCH):
            Ga = gatha.tile([P, GPC, 2 * C], BF16, tag="ga")
            if INDIRECT:
                nc.gpsimd.indirect_dma_start(
                    out=Ga[:].rearrange("p g (r c) -> p (g r) c", r=2),
                    out_offset=None,
                    in_=featT.ap(),
                    in_offset=IndirectOffsetOnAxis(
                        ap=aidx[:, ch * GPC:(ch + 1) * GPC, :], axis=0))
            else:
                for s in range(NI // 1024):
                    f0 = ch * (NI // 16) + s * 64
                    nc.gpsimd.dma_gather(
                        out_ap=Ga[:, s * 8:(s + 1) * 8, :], in_ap=gsrc,
                        idxs_ap=wrapA[:, 0, f0:f0 + 64],
                        num_idxs=1024, num_idxs_reg=1024, elem_size=2 * C,
                        elem_step=C, queue_num=next_q())
            Gas.append(Ga)

        # ---------------- software-pipelined chunk loop ----------------
        # fe(ch) (which issues ch's deform gathers) is emitted before
        # combine(ch-1) so gathers stay a chunk ahead of the combines.
        pend = [None]

        def front_end(ch):
            g0 = ch * GPC
            Ga = Gas[ch]

            # lerp: d = f1 - f0 (bf16 2x); fa = frac*d + f0 per g-column
            d = fab.tile([P, GPC, C], BF16, tag="dl")
            nc.vector.tensor_tensor(out=d[:], in0=Ga[:, :, C:2 * C],
                                    in1=Ga[:, :, 0:C], op=Alu.subtract)
            rin0 = rpool.tile([P, NI], BF16, tag="rin0")
            rin1 = rpool.tile([P, NI], BF16, tag="rin1")
            for gi in range(GPC):
                g = g0 + gi
                fa = fab.tile([P, C], BF16, tag="fa")
                nc.vector.scalar_tensor_tensor(
                    out=fa[:], in0=d[:, gi, :], scalar=fraca[:, g:g + 1],
                    in1=Ga[:, gi, 0:C], op0=Alu.mult, op1=Alu.add)
                for hh in range(2):
                    tp = tps.tile([P, P], BF16, tag="tp", space="PSUM")
                    nc.tensor.transpose(out=tp[:],
                                        in_=fa[:, hh * 128:(hh + 1) * 128],
                                        identity=ident[:])
                    rdst = (rin0 if hh == 0 else rin1)
                    nc.scalar.copy(out=rdst[:, gi * 128:(gi + 1) * 128],
                                   in_=tp[:])

            # MLP (bf16): h = lrelu(rin@W1 + b1); g = lrelu(h + h@Wr + br)
            hb = rpool.tile([H, NI], BF16, tag="hb")
            gg = rpool.tile([H, NI], BF16, tag="gg")
            for n in range(NI // 512):
                sl = slice(n * 512, (n + 1) * 512)
                gsl = slice(ch * NI + n * 512, ch * NI + (n + 1) * 512)
                ps1 = mmps.tile([H, 512], F32, tag="ps1", space="PSUM")
                nc.tensor.matmul(out=ps1[:], lhsT=w1a0_sb[:], rhs=rin0[:, sl],
                                 start=True, stop=False)
                nc.tensor.matmul(out=ps1[:], lhsT=w1a1_sb[:], rhs=rin1[:, sl],
                                 start=False, stop=False)
                nc.tensor.matmul(out=ps1[:], lhsT=wxc_sb[:], rhs=xcb_sb[:, gsl],
                                 start=False, stop=True)
                def lrelu(dst, ps, bias_sb, tag):
                    if not SIM_SAFE:
                        nc.scalar.activation(out=dst, in_=ps, func=Act.Prelu,
                                             bias=bias_sb[:, :], scale=1.0,
                                             alpha=0.2)
                    else:
                        t = sc.tile([H, 512], F32, tag=tag)
                        nc.scalar.activation(out=t[:], in_=ps,
                                             func=Act.Identity,
                                             bias=bias_sb[:, :], scale=1.0)
                        nc.vector.scalar_tensor_tensor(
                            out=dst, in0=t[:], scalar=0.2, in1=t[:],
                            op0=Alu.mult, op1=Alu.max)

                lrelu(hb[:, sl], ps1[:], b1_sb, "lr1")
                ps2 = mmps.tile([H, 512], F32, tag="ps1", space="PSUM")
                nc.tensor.matmul(out=ps2[:], lhsT=wr1_sb[:], rhs=hb[:, sl],
                                 start=True, stop=True)
                lrelu(gg[:, sl], ps2[:], br_sb, "lr2")

            # out3 = g@W3 (+ b3 on DVE), query-major [P, GPC, 12]
            o3 = rpool.tile([P, GPC, 12], F32, tag="o3")
            for gi in range(GPC):
                ps3 = l3ps.tile([P, 12], F32, tag="ps3", space="PSUM")
                nc.tensor.matmul(out=ps3[:],
                                 lhsT=gg[:, gi * 128:(gi + 1) * 128],
                                 rhs=w3_sb[:], start=True, stop=True)
                nc.scalar.copy(out=o3[:, gi, :], in_=ps3[:])
            nc.vector.tensor_tensor(out=o3[:], in0=o3[:],
                                    in1=_bc_mid(b3_sb[:], GPC), op=Alu.add)

            # ---- scalar stage (tiles [P, GPC] / [P, GK] f32) ----
            def softplus(dst, src_ap, tag):
                a = sc.tile([P, GPC], F32, tag=tag + "a")
                nc.scalar.activation(out=a[:], in_=src_ap, func=Act.Abs)
                e = sc.tile([P, GPC], F32, tag=tag + "e")
                nc.scalar.activation(out=e[:], in_=a[:], func=Act.Exp,
                                     scale=-1.0)
                lg = sc.tile([P, GPC], F32, tag=tag + "l")
                nc.scalar.activation(out=lg[:], in_=e[:], func=Act.Ln,
                                     bias=1.0, scale=1.0)
                m = sc.tile([P, GPC], F32, tag=tag + "m")
                nc.vector.tensor_scalar(out=m[:], in0=src_ap, scalar1=0.0,
                                        scalar2=None, op0=Alu.max)
                nc.vector.tensor_tensor(out=dst, in0=lg[:], in1=m[:],
                                        op=Alu.add)

            r_t = sc.tile([P, GPC], F32, tag="rt")
            softplus(r_t[:], o3[:, :, 0], "spr")
            nc.vector.tensor_scalar(out=r_t[:], in0=r_t[:], scalar1=0.3,
                                    scalar2=2.0, op0=Alu.add, op1=Alu.min)
            sg_t = sc.tile([P, GPC], F32, tag="sgt")
            softplus(sg_t[:], o3[:, :, 1], "sps")
            nc.vector.tensor_scalar(out=sg_t[:], in0=sg_t[:], scalar1=0.5,
                                    scalar2=3.0, op0=Alu.add, op1=Alu.min)
            s2 = sc.tile([P, GPC], F32, tag="s2")
            nc.vector.tensor_tensor(out=s2[:], in0=sg_t[:], in1=sg_t[:],
                                    op=Alu.mult)
            nc.vector.tensor_scalar(out=s2[:], in0=s2[:], scalar1=4.0,
                                    scalar2=1e-8, op0=Alu.mult, op1=Alu.add)
            rs = sc.tile([P, GPC], F32, tag="rs")
            nc.vector.reciprocal(out=rs[:], in_=s2[:])

            # tanh(res_raw) via exp: 1 - 2/(exp(2x)+1)
            th = sc.tile([P, GK], F32, tag="th")
            nc.scalar.activation(out=th[:], in_=o3[:, :, 2:7], func=Act.Exp,
                                 scale=2.0)
            nc.vector.tensor_scalar(out=th[:], in0=th[:], scalar1=1.0,
                                    scalar2=None, op0=Alu.add)
            nc.vector.reciprocal(out=th[:], in_=th[:])
            nc.vector.tensor_scalar(out=th[:], in0=th[:], scalar1=-2.0,
                                    scalar2=1.0, op0=Alu.mult, op1=Alu.add)
            # sigmoid(gate_raw) via exp: 1/(exp(-x)+1)
            gt = sc.tile([P, GK], F32, tag="gt")
            nc.scalar.activation(out=gt[:], in_=o3[:, :, 7:12], func=Act.Exp,
                                 scale=-1.0)
            nc.vector.tensor_scalar(out=gt[:], in0=gt[:], scalar1=1.0,
                                    scalar2=None, op0=Alu.add)
            nc.vector.reciprocal(out=gt[:], in_=gt[:])

            off = sc.tile([P, GK], F32, tag="off")
            nc.vector.tensor_tensor(out=off[:], in0=_bc(r_t[:], K),
                                    in1=_bc_mid(base_sb[:], GPC), op=Alu.mult)
            nc.vector.scalar_tensor_tensor(out=off[:], in0=th[:], scalar=0.5,
                                           in1=off[:], op0=Alu.mult,
                                           op1=Alu.add)
            dix = sc.tile([P, GK], F32, tag="dix")
            nc.vector.scalar_tensor_tensor(
                out=dix[:], in0=off[:], scalar=float(DXSCALE),
                in1=_bc(xq[:, g0:g0 + GPC], K), op0=Alu.mult, op1=Alu.add)
            nc.vector.tensor_scalar(out=dix[:], in0=dix[:], scalar1=1.0,
                                    scalar2=0.5, op0=Alu.add, op1=Alu.mult)
            nc.vector.tensor_scalar(out=dix[:], in0=dix[:],
                                    scalar1=float(IXSCALE), scalar2=0.0,
                                    op0=Alu.mult, op1=Alu.max)
            nc.vector.tensor_scalar(out=dix[:], in0=dix[:],
                                    scalar1=float(IXSCALE), scalar2=None,
                                    op0=Alu.min)
            fracd = sc.tile([P, GK], F32, tag="fracd")
            i0fd = sc.tile([P, GK], F32, tag="i0fd")
            ti_d = sc.tile([P, GK], I32, tag="tid")
            nc.vector.tensor_copy(out=ti_d[:], in_=dix[:])
            nc.vector.tensor_copy(out=i0fd[:], in_=ti_d[:])
            gt_d = sc.tile([P, GK], F32, tag="gtd")
            nc.vector.tensor_tensor(out=gt_d[:], in0=i0fd[:], in1=dix[:],
                                    op=Alu.is_gt)
            nc.vector.tensor_tensor(out=i0fd[:], in0=i0fd[:], in1=gt_d[:],
                                    op=Alu.subtract)
            nc.vector.tensor_scalar(out=i0fd[:], in0=i0fd[:],
                                    scalar1=float(L - 2), scalar2=None,
                                    op0=Alu.min)
            nc.vector.tensor_tensor(out=fracd[:], in0=dix[:], in1=i0fd[:],
                                    op=Alu.subtract)

            o2 = sc.tile([P, GK], F32, tag="o2")
            nc.vector.tensor_tensor(out=o2[:], in0=off[:], in1=off[:],
                                    op=Alu.mult)
            nc.vector.tensor_tensor(out=o2[:], in0=o2[:], in1=_bc(rs[:], K),
                                    op=Alu.mult)
            w_t = sc.tile([P, GK], F32, tag="wt")
            nc.scalar.activation(out=w_t[:], in_=o2[:], func=Act.Exp,
                                 scale=-0.5)
            nc.vector.tensor_tensor(out=w_t[:], in0=w_t[:], in1=gt[:],
                                    op=Alu.mult)
            wsum = sc.tile([P, GPC], F32, tag="wsum")
            w_v = w_t[:].rearrange("p (g k) -> p g k", k=K)
            nc.vector.tensor_reduce(out=wsum[:], in_=w_v,
                                    axis=mybir.AxisListType.X, op=Alu.add)
            nc.vector.tensor_scalar(out=wsum[:], in0=wsum[:], scalar1=1e-8,
                                    scalar2=None, op0=Alu.add)
            rn = sc.tile([P, GPC], F32, tag="rn")
            nc.vector.reciprocal(out=rn[:], in_=wsum[:])
            wn = sc.tile([P, GK], F32, tag="wn")
            nc.vector.tensor_tensor(out=wn[:], in0=w_t[:], in1=_bc(rn[:], K),
                                    op=Alu.mult)
            c1 = sc.tile([P, GK], F32, tag="c1")
            nc.vector.tensor_tensor(out=c1[:], in0=wn[:], in1=fracd[:],
                                    op=Alu.mult)
            c0 = sc.tile([P, GK], F32, tag="c0")
            nc.vector.tensor_tensor(out=c0[:], in0=wn[:], in1=c1[:],
                                    op=Alu.subtract)

            if INDIRECT:
                # deform offset pairs [P, GK, 2] i32 = [i0; i0+1]
                didx = sc.tile([P, GK, 2], I32, tag="didx")
                i0p1 = sc.tile([P, GK], F32, tag="i0p1")
                nc.vector.tensor_scalar(out=i0p1[:], in0=i0fd[:], scalar1=1.0,
                                        scalar2=None, op0=Alu.add)
                nc.vector.tensor_copy(out=didx[:, :, 0], in_=i0fd[:])
                nc.vector.tensor_copy(out=didx[:, :, 1], in_=i0p1[:])

                # one indirect gather for all 5 taps' row-pairs of this chunk
                Gd = gathd.tile([P, GK, 2 * C], BF16, tag="gd")
                nc.gpsimd.indirect_dma_start(
                    out=Gd[:].rearrange("p n (r c) -> p (n r) c", r=2),
                    out_offset=None,
                    in_=featT.ap(),
                    in_offset=IndirectOffsetOnAxis(ap=didx[:], axis=0))

                def gsl(k, gi, half):
                    n = gi * K + k
                    return Gd[:, n, half * C:(half + 1) * C]
            else:
                wrepD = wdp.tile([P, K, NI // 16], I16, tag="wd")
                wrapped_idx(i0fd[:], K, wrepD)
                Gds = []
                for k in range(K):
                    Gd = gathd.tile([P, GPC, 2 * C], BF16, tag="gd")
                    nc.gpsimd.dma_gather(
                        out_ap=Gd[:], in_ap=gsrc,
                        idxs_ap=wrepD[:, k, :], num_idxs=NI, num_idxs_reg=NI,
                        elem_size=2 * C, elem_step=C, queue_num=next_q())
                    Gds.append(Gd)

                def gsl(k, gi, half):
                    return Gds[k][:, gi, half * C:(half + 1) * C]
            return gsl, c0, c1

        def combine_and_store(gsl, c0, c1, ch):
            accV = accp.tile([P, GPC, C], BF16 if ACT_OFFLOAD else F32,
                             tag="accV")
            for gi in range(GPC):
                acc = accV[:, gi, :]
                if ACT_OFFLOAD and gi < ACT_GIS:
                    # ACT: prod = row * c (per-partition scale); DVE: bf16
                    # tensor_tensor adds (2x mode) reduce the 10 products.
                    prods = []
                    for k in range(K):
                        n = gi * K + k
                        for half, cw in ((0, c0), (1, c1)):
                            pr = prodp.tile([P, C], BF16, tag="pr")
                            nc.scalar.activation(
                                out=pr[:], in_=gsl(k, gi, half), func=Act.Copy,
                                scale=cw[:, n:n + 1])
                            prods.append(pr)
                    nc.vector.tensor_tensor(out=acc, in0=prods[0][:],
                                            in1=prods[1][:], op=Alu.add)
                    for pr in prods[2:]:
                        nc.vector.tensor_tensor(out=acc, in0=acc, in1=pr[:],
                                                op=Alu.add)
                    continue
                for k in range(K):
                    n = gi * K + k
                    csc0 = c0[:, n:n + 1]
                    csc1 = c1[:, n:n + 1]
                    if k == 0:
                        nc.vector.tensor_scalar(out=acc, in0=gsl(k, gi, 0),
                                                scalar1=csc0, scalar2=None,
                                                op0=Alu.mult)
                    else:
                        nc.vector.scalar_tensor_tensor(
                            out=acc, in0=gsl(k, gi, 0), scalar=csc0, in1=acc,
                            op0=Alu.mult, op1=Alu.add)
                    nc.vector.scalar_tensor_tensor(
                        out=acc, in0=gsl(k, gi, 1), scalar=csc1, in1=acc,
                        op0=Alu.mult, op1=Alu.add)
            outv = out.ap().rearrange("(g p) c -> p g c", p=P)
            g0 = ch * GPC
            nc.sync.dma_start(out=outv[:, g0:g0 + GPC, :], in_=accV[:])

        for ch in range(NCH):
            args = front_end(ch)
            if pend[0] is not None:
                combine_and_store(*pend[0])
            pend[0] = (*args, ch)
        combine_and_store(*pend[0])


_PROGRAM = None


def _get_program():
    global _PROGRAM
    if _PROGRAM is None:
        _PROGRAM = build_program()
    return _PROGRAM


def make_in_maps(feat_1d, coords_1d, cell_1d, W1, b1, Wr, br, W3, b3):
    """Build the 8 per-core input dicts from full inputs."""
    from ml_dtypes import bfloat16
    f32 = np.float32
    W1 = np.asarray(W1, f32)
    wr1 = (np.asarray(Wr, f32) + np.eye(H, dtype=f32)).astype(bfloat16)
    base = np.array([-2.0, -1.0, 0.0, 1.0, 2.0], f32)
    base128 = np.broadcast_to(base, (P, K)).copy()
    b3rep = np.broadcast_to(np.asarray(b3, f32), (P, 12)).copy()
    sel = np.zeros((P, 8, 128), f32)
    for a in range(8):
        for m in range(128):
            sel[16 * a + m % 16, a, m] = 1.0
    shared = {
        "sel8": sel.reshape(P, 8 * 128),
        "w1a0": np.ascontiguousarray(W1[0:128]).astype(bfloat16),
        "w1a1": np.ascontiguousarray(W1[128:256]).astype(bfloat16),
        "wxc": np.ascontiguousarray(W1[256:258]).astype(bfloat16),
        "b1c": np.asarray(b1, f32).reshape(H, 1).copy(),
        "wr1": wr1,
        "brc": np.asarray(br, f32).reshape(H, 1).copy(),
        "w3c": np.asarray(W3, f32).astype(bfloat16),
        "b3rep": b3rep,
        "base128": base128,
    }
    featTs = [np.ascontiguousarray(np.asarray(feat_1d[b], f32).T)
              .astype(bfloat16) for b in range(B)]
    in_maps = []
    for core in range(NCORES):
        b = core // 2
        s = core % 2
        sl = slice(s * Q, (s + 1) * Q)
        cds = np.ascontiguousarray(np.asarray(coords_1d[b, sl, 0], f32))
        cel = np.ascontiguousarray(np.asarray(cell_1d[b, sl, 0], f32))
        xcb = np.stack([cds, cel]).astype(bfloat16)
        in_maps.append({
            "featT": featTs[b],
            "coords": cds,
            "xcb": xcb,
            **shared,
        })
    return in_maps


def kernel(feat_1d, coords_1d, cell_1d, W1, b1, Wr, br, W3, b3):
    from concourse.bass_utils import run_bass_kernel_spmd
    nc = _get_program()
    in_maps = make_in_maps(feat_1d, coords_1d, cell_1d, W1, b1, Wr, br, W3, b3)
    res = run_bass_kernel_spmd(nc, in_maps, core_ids=list(range(NCORES)))
    outf = np.zeros((B, N, C), np.float32)
    for core in range(NCORES):
        b = core // 2
        s = core % 2
        outf[b, s * Q:(s + 1) * Q, :] = np.asarray(
            res.results[core]["out"]).astype(np.float32)
    return outf


# revision 64
# speedup vs baseline: 1.0888x; 1.0019x over previous
"""Trainium2 Bass kernel for nn_DeformableDynamicGather1D.

Sharding: 8 cores = 4 batches x 2 query-halves. Each core handles one batch's
feat and Q=4096 queries.

Host prep: feat is transposed to feat_T [L, C] and cast to bf16 on the host
(layout/precision prep, same class as the existing weight repacking). Router
weights are pre-cast to bf16; Wr has identity folded in; b3 is replicated to
[128, 12]; coords/cell are pre-cast to a bf16 [2, Q] tile for the MLP tail.

Device pipeline, per 1024-query chunk (4 chunks, software-pipelined so chunk
N's gathers run while chunk N-1 combines):

  1. Anchor: bilinear indices from coords (f32 DVE math, explicit [i0; i0+1]
     int32 offset pairs); indirect_dma_start row gather from feat_T (hardware
     dynamic DGE -- the DMA engine reads the offset table, Pool is not
     blocked); lerp on DVE (bf16); PE-transpose into channel-major rin.
  2. MLP on PE in bf16 (1 cycle/row): h = lrelu(rin@W1+b1) via ACT Prelu;
     g = lrelu(h@(Wr+I)+br); out3 = g@W3 + b3 (b3 added on DVE).
  3. Scalar stage: softplus via Abs/Exp/Ln, tanh/sigmoid via Exp + DVE
     reciprocal -- every ACT func lives in the natural_log_exp table, so no
     act-table reloads. Produces deform offset pairs and weights c0/c1.
  4. Deform: one indirect gather per chunk fetches all 5 taps' row-pairs
     (bf16); combine with scalar_tensor_tensor FMAs on DVE into a f32
     accumulator; per-chunk output DMA overlaps the next chunk's compute.

Query <-> tile coordinates: q = g*128 + p (tile [128 p, G g]).
"""
import os
import sys

for _p in ("/opt/trn_rl_repo", "/root/.axon_site/_ro/trn_rl_repo"):
    if os.path.isdir(_p) and _p not in sys.path:
        sys.path.append(_p)

import numpy as np
import concourse.bass as bass
import concourse.bacc as bacc
import concourse.tile as tile
from concourse import mybir
from concourse.bass import AP, IndirectOffsetOnAxis
from concourse.masks import make_identity

F32 = mybir.dt.float32
BF16 = mybir.dt.bfloat16
I32 = mybir.dt.int32
Act = mybir.ActivationFunctionType
Alu = mybir.AluOpType

P = 128          # partitions
G = 32           # q = g*128 + p
Q = P * G        # 4096 queries per core
C = 256          # channels
L = 4096         # feat length
H = 64           # hidden
K = 5            # taps
NCORES = 8
B, N = 4, 8192   # full problem
NI = int(os.environ.get("KERNEL_NI", "1024"))  # queries per chunk
NCH = Q // NI    # chunks
GPC = NI // P    # g-columns per chunk
GK = GPC * K

IXSCALE = np.float32(float(L - 1))          # 4095
DXSCALE = np.float32(2.0 / max(L - 1, 1))   # reference scale_x

# CoreSim lacks Prelu: sim-safe mode uses Identity + DVE leaky instead
# (numerically identical; only used by the local debug harness).
SIM_SAFE = os.environ.get("KERNEL_SIM_SAFE") == "1"

# Gather engine: per-tap blocking dma_gather with the wrapped-i16 idx layout
# (default), vs indirect_dma_start (hardware dynamic DGE -- passes CoreSim
# but aborts at runtime on this hardware stack; kept for reference).
INDIRECT = os.environ.get("KERNEL_GATHER", "dma_gather") == "indirect"


def _bc(ap2d: AP, extra: int) -> AP:
    """Broadcast a [p, n] AP to [p, n, extra] with stride-0 inner dim."""
    return AP(tensor=ap2d.tensor, offset=ap2d.offset,
              ap=[*ap2d.ap, [0, extra]])


def _bc_mid(ap2d: AP, mid: int) -> AP:
    """Broadcast a [p, n] AP to [p, mid, n] with stride-0 middle dim."""
    return AP(tensor=ap2d.tensor, offset=ap2d.offset,
              ap=[ap2d.ap[0], [0, mid], ap2d.ap[1]])


# Multi-queue SWDGE is unsupported under Tile: DMASW lane sems get locked to
# the first queue that uses them and the scheduler's lane rotation cannot be
# aligned with a per-call queue rotation (CoreSim flags the conflict).
NQUEUES = int(os.environ.get("KERNEL_NQ", "1"))

# Offload most combine FMAs to the Scalar engine: ACT computes the
# per-partition-scaled products (out = gathered_row * c), DVE reduces them
# with bf16 tensor_tensor adds (2x mode) into bf16 accumulators; output
# tensor becomes bf16 and the host converts to f32. g-columns >= ACT_GIS
# stay on the DVE scalar_tensor_tensor path.
ACT_OFFLOAD = os.environ.get("KERNEL_ACTOFF", "1") == "1"
ACT_GIS = 6


def build_program():
    nc = bacc.Bacc("TRN2", target_bir_lowering=False, debug=False,
                   num_devices=NCORES, num_swdge_queues=NQUEUES)

    featT = nc.dram_tensor("featT", [L, C], BF16, kind="ExternalInput")
    coords = nc.dram_tensor("coords", [Q], F32, kind="ExternalInput")
    xcb = nc.dram_tensor("xcb", [2, Q], BF16, kind="ExternalInput")
    w1a0 = nc.dram_tensor("w1a0", [128, H], BF16, kind="ExternalInput")
    w1a1 = nc.dram_tensor("w1a1", [128, H], BF16, kind="ExternalInput")
    wxc = nc.dram_tensor("wxc", [2, H], BF16, kind="ExternalInput")
    b1c = nc.dram_tensor("b1c", [H, 1], F32, kind="ExternalInput")
    wr1 = nc.dram_tensor("wr1", [H, H], BF16, kind="ExternalInput")
    brc = nc.dram_tensor("brc", [H, 1], F32, kind="ExternalInput")
    w3c = nc.dram_tensor("w3c", [H, 12], BF16, kind="ExternalInput")
    b3rep = nc.dram_tensor("b3rep", [P, 12], F32, kind="ExternalInput")
    base128 = nc.dram_tensor("base128", [P, K], F32, kind="ExternalInput")
    sel8 = nc.dram_tensor("sel8", [P, 8 * 128], F32, kind="ExternalInput")
    out = nc.dram_tensor("out", [Q, C], BF16 if ACT_OFFLOAD else F32,
                         kind="ExternalOutput")

    with tile.TileContext(nc) as tc:
        _body(nc, tc, featT, coords, xcb, w1a0, w1a1, wxc, b1c, wr1, brc,
              w3c, b3rep, base128, sel8, out)
    nc.compile()
    return nc


def _body(nc, tc, featT, coords, xcb, w1a0, w1a1, wxc, b1c, wr1, brc,
          w3c, b3rep, base128, sel8, out):
    import contextlib
    ctx = contextlib.ExitStack()
    with ctx:
        big = NI > 1024   # larger chunks: shrink buffer rings to fit SBUF
        const = ctx.enter_context(tc.tile_pool(name="const", bufs=1))
        rpool = ctx.enter_context(tc.tile_pool(name="rpool", bufs=1 if big else 2))
        gatha = ctx.enter_context(tc.tile_pool(name="gatha", bufs=1 if big else 4))
        gathd = ctx.enter_context(
            tc.tile_pool(name="gathd", bufs=2 if INDIRECT else (6 if big else 10)))
        fab = ctx.enter_context(tc.tile_pool(name="fab", bufs=1 if big else 2))
        sc = ctx.enter_context(tc.tile_pool(name="scal", bufs=3))
        wdp = ctx.enter_context(tc.tile_pool(name="wdp", bufs=2))
        accp = ctx.enter_context(tc.tile_pool(name="accp", bufs=1 if big else 2))
        prodp = ctx.enter_context(tc.tile_pool(name="prodp", bufs=12))
        tps = ctx.enter_context(tc.tile_pool(name="tps", bufs=2, space="PSUM"))
        mmps = ctx.enter_context(tc.tile_pool(name="mmps", bufs=2, space="PSUM"))
        l3ps = ctx.enter_context(tc.tile_pool(name="l3ps", bufs=2, space="PSUM"))

        ident = const.tile([P, P], BF16)
        make_identity(nc, ident[:])

        # weights / constants
        w1a0_sb = const.tile([128, H], BF16)
        w1a1_sb = const.tile([128, H], BF16)
        wxc_sb = const.tile([2, H], BF16)
        b1_sb = const.tile([H, 1], F32)
        wr1_sb = const.tile([H, H], BF16)
        br_sb = const.tile([H, 1], F32)
        w3_sb = const.tile([H, 12], BF16)
        b3_sb = const.tile([P, 12], F32)
        base_sb = const.tile([P, K], F32)
        xcb_sb = const.tile([2, Q], BF16)
        loads = [(w1a0_sb, w1a0), (w1a1_sb, w1a1), (wxc_sb, wxc),
                 (b1_sb, b1c), (wr1_sb, wr1), (br_sb, brc),
                 (w3_sb, w3c), (b3_sb, b3rep), (base_sb, base128),
                 (xcb_sb, xcb)]
        if not INDIRECT:
            sel_sb = const.tile([P, 8 * 128], F32)
            loads.append((sel_sb, sel8))
        for dst, src in loads:
            nc.sync.dma_start(out=dst[:], in_=src.ap())

        # feat_T row-pair view for dma_gather: idx i -> elems [256*i, +512)
        gsrc = AP(tensor=featT.ap().tensor, offset=0,
                  ap=[[C, L - 1], [1, 2 * C]])
        I16 = mybir.dt.int16
        qctr = [0]

        def next_q():
            q = qctr[0] % NQUEUES
            qctr[0] += 1
            return q

        def wrapped_idx(vf32_ap, nk, wrep):
            """Build replicated wrapped int16 idx tile from a query-major f32
            index tile V [128, nk*Gq] ((g, k)-major cols: n = g*nk + k) via 8
            selection matmuls; wrep is [128, nk, (128*Gq)//16] i16."""
            Gq = vf32_ap.shape[-1] // nk
            for a in range(8):
                psw = l3ps.tile([P, GK], F32, tag="psw", space="PSUM")
                nc.tensor.matmul(
                    out=psw[:, 0:nk * Gq], lhsT=sel_sb[:, a * 128:(a + 1) * 128],
                    rhs=vf32_ap, start=True, stop=True)
                dst = AP(tensor=wrep[:].tensor, offset=wrep[:].offset + a,
                         ap=[wrep[:].ap[0], [8 * Gq, nk], [8, Gq]])
                src = AP(tensor=psw[:].tensor, offset=psw[:].offset,
                         ap=[psw[:].ap[0], [1, nk], [nk, Gq]])
                nc.vector.tensor_copy(out=dst, in_=src)

        # ---- anchor index math, full Q upfront (query-major [P, G]) ----
        xq = const.tile([P, G], F32)
        nc.sync.dma_start(
            out=xq[:],
            in_=AP(tensor=coords.ap().tensor, offset=0, ap=[[1, P], [P, G]]))
        ixf = const.tile([P, G], F32)
        nc.vector.tensor_scalar(out=ixf[:], in0=xq[:], scalar1=1.0,
                                scalar2=0.5, op0=Alu.add, op1=Alu.mult)
        nc.vector.tensor_scalar(out=ixf[:], in0=ixf[:], scalar1=float(IXSCALE),
                                scalar2=0.0, op0=Alu.mult, op1=Alu.max)
        nc.vector.tensor_scalar(out=ixf[:], in0=ixf[:], scalar1=float(IXSCALE),
                                scalar2=None, op0=Alu.min)
        # i0 = min(floor(ix), L-2); frac = ix - i0 (floor via int convert +
        # fixup, correct for both trunc and round-nearest convert modes)
        fraca = const.tile([P, G], F32)
        i0fa = const.tile([P, G], F32)
        ti_a = const.tile([P, G], I32)
        nc.vector.tensor_copy(out=ti_a[:], in_=ixf[:])
        nc.vector.tensor_copy(out=i0fa[:], in_=ti_a[:])
        gt_a = const.tile([P, G], F32)
        nc.vector.tensor_tensor(out=gt_a[:], in0=i0fa[:], in1=ixf[:],
                                op=Alu.is_gt)
        nc.vector.tensor_tensor(out=i0fa[:], in0=i0fa[:], in1=gt_a[:],
                                op=Alu.subtract)
        nc.vector.tensor_scalar(out=i0fa[:], in0=i0fa[:], scalar1=float(L - 2),
                                scalar2=None, op0=Alu.min)
        nc.vector.tensor_tensor(out=fraca[:], in0=ixf[:], in1=i0fa[:],
                                op=Alu.subtract)
        if INDIRECT:
            # anchor offset pairs [P, G, 2] i32 = [i0; i0+1]
            aidx = const.tile([P, G, 2], I32)
            i0p1a = const.tile([P, G], F32)
            nc.vector.tensor_scalar(out=i0p1a[:], in0=i0fa[:], scalar1=1.0,
                                    scalar2=None, op0=Alu.add)
            nc.vector.tensor_copy(out=aidx[:, :, 0], in_=i0fa[:])
            nc.vector.tensor_copy(out=aidx[:, :, 1], in_=i0p1a[:])
        else:
            wrapA = const.tile([P, 1, Q // 16], I16)
            wrapped_idx(i0fa[:], 1, wrapA)

        # All anchor gathers issued upfront: each chunk's MLP/scalar chain
        # (which gates its deform gathers) then overlaps the previous chunk's
        # deform burst instead of queueing its anchor behind it on Pool.
        Gas = []
        for ch in range(NCH):
            Ga = gatha.tile([P, GPC, 2 * C], BF16, tag="ga")
            if INDIRECT:
                nc.gpsimd.indirect_dma_start(
                    out=Ga[:].rearrange("p g (r c) -> p (g r) c", r=2),
                    out_offset=None,
                    in_=featT.ap(),
                    in_offset=IndirectOffsetOnAxis(
                        ap=aidx[:, ch * GPC:(ch + 1) * GPC, :], axis=0))
            else:
                for s in range(NI // 1024):
                    f0 = ch * (NI // 16) + s * 64
                    nc.gpsimd.dma_gather(
                        out_ap=Ga[:, s * 8:(s + 1) * 8, :], in_ap=gsrc,
                        idxs_ap=wrapA[:, 0, f0:f0 + 64],
                        num_idxs=1024, num_idxs_reg=1024, elem_size=2 * C,
                        elem_step=C, queue_num=next_q())
            Gas.append(Ga)

        # ---------------- software-pipelined chunk loop ----------------
        # fe(ch) (which issues ch's deform gathers) is emitted before
        # combine(ch-1) so gathers stay a chunk ahead of the combines.
        pend = [None]

        def front_end(ch):
            g0 = ch * GPC
            Ga = Gas[ch]

            # lerp: d = f1 - f0 (bf16 2x); fa = frac*d + f0 per g-column
            d = fab.tile([P, GPC, C], BF16, tag="dl")
            nc.vector.tensor_tensor(out=d[:], in0=Ga[:, :, C:2 * C],
                                    in1=Ga[:, :, 0:C], op=Alu.subtract)
            rin0 = rpool.tile([P, NI], BF16, tag="rin0")
            rin1 = rpool.tile([P, NI], BF16, tag="rin1")
            for gi in range(GPC):
                g = g0 + gi
                fa = fab.tile([P, C], BF16, tag="fa")
                nc.vector.scalar_tensor_tensor(
                    out=fa[:], in0=d[:, gi, :], scalar=fraca[:, g:g + 1],
                    in1=Ga[:, gi, 0:C], op0=Alu.mult, op1=Alu.add)
                for hh in range(2):
                    tp = tps.tile([P, P], BF16, tag="tp", space="PSUM")
                    nc.tensor.transpose(out=tp[:],
                                        in_=fa[:, hh * 128:(hh + 1) * 128],
                                        identity=ident[:])
                    rdst = (rin0 if hh == 0 else rin1)
                    nc.scalar.copy(out=rdst[:, gi * 128:(gi + 1) * 128],
                                   in_=tp[:])

            # MLP (bf16): h = lrelu(rin@W1 + b1); g = lrelu(h + h@Wr + br)
            hb = rpool.tile([H, NI], BF16, tag="hb")
            gg = rpool.tile([H, NI], BF16, tag="gg")
            for n in range(NI // 512):
                sl = slice(n * 512, (n + 1) * 512)
                gsl = slice(ch * NI + n * 512, ch * NI + (n + 1) * 512)
                ps1 = mmps.tile([H, 512], F32, tag="ps1", space="PSUM")
                nc.tensor.matmul(out=ps1[:], lhsT=w1a0_sb[:], rhs=rin0[:, sl],
                                 start=True, stop=False)
                nc.tensor.matmul(out=ps1[:], lhsT=w1a1_sb[:], rhs=rin1[:, sl],
                                 start=False, stop=False)
                nc.tensor.matmul(out=ps1[:], lhsT=wxc_sb[:], rhs=xcb_sb[:, gsl],
                                 start=False, stop=True)
                def lrelu(dst, ps, bias_sb, tag):
                    if not SIM_SAFE:
                        nc.scalar.activation(out=dst, in_=ps, func=Act.Prelu,
                                             bias=bias_sb[:, :], scale=1.0,
                                             alpha=0.2)
                    else:
                        t = sc.tile([H, 512], F32, tag=tag)
                        nc.scalar.activation(out=t[:], in_=ps,
                                             func=Act.Identity,
                                             bias=bias_sb[:, :], scale=1.0)
                        nc.vector.scalar_tensor_tensor(
                            out=dst, in0=t[:], scalar=0.2, in1=t[:],
                            op0=Alu.mult, op1=Alu.max)

                lrelu(hb[:, sl], ps1[:], b1_sb, "lr1")
                ps2 = mmps.tile([H, 512], F32, tag="ps1", space="PSUM")
                nc.tensor.matmul(out=ps2[:], lhsT=wr1_sb[:], rhs=hb[:, sl],
                                 start=True, stop=True)
                lrelu(gg[:, sl], ps2[:], br_sb, "lr2")

            # out3 = g@W3 (+ b3 on DVE), query-major [P, GPC, 12]
            o3 = rpool.tile([P, GPC, 12], F32, tag="o3")
            for gi in range(GPC):
                ps3 = l3ps.tile([P, 12], F32, tag="ps3", space="PSUM")
                nc.tensor.matmul(out=ps3[:],
                                 lhsT=gg[:, gi * 128:(gi + 1) * 128],
                                 rhs=w3_sb[:], start=True, stop=True)
                nc.scalar.copy(out=o3[:, gi, :], in_=ps3[:])
            nc.vector.tensor_tensor(out=o3[:], in0=o3[:],
                                    in1=_bc_mid(b3_sb[:], GPC), op=Alu.add)

            # ---- scalar stage (tiles [P, GPC] / [P, GK] f32) ----
            def softplus(dst, src_ap, tag):
                a = sc.tile([P, GPC], F32, tag=tag + "a")
                nc.scalar.activation(out=a[:], in_=src_ap, func=Act.Abs)
                e = sc.tile([P, GPC], F32, tag=tag + "e")
                nc.scalar.activation(out=e[:], in_=a[:], func=Act.Exp,
                                     scale=-1.0)
                lg = sc.tile([P, GPC], F32, tag=tag + "l")
                nc.scalar.activation(out=lg[:], in_=e[:], func=Act.Ln,
                                     bias=1.0, scale=1.0)
                m = sc.tile([P, GPC], F32, tag=tag + "m")
                nc.vector.tensor_scalar(out=m[:], in0=src_ap, scalar1=0.0,
                                        scalar2=None, op0=Alu.max)
                nc.vector.tensor_tensor(out=dst, in0=lg[:], in1=m[:],
                                        op=Alu.add)

            r_t = sc.tile([P, GPC], F32, tag="rt")
            softplus(r_t[:], o3[:, :, 0], "spr")
            nc.vector.tensor_scalar(out=r_t[:], in0=r_t[:], scalar1=0.3,
                                    scalar2=2.0, op0=Alu.add, op1=Alu.min)
            sg_t = sc.tile([P, GPC], F32, tag="sgt")
            softplus(sg_t[:], o3[:, :, 1], "sps")
            nc.vector.tensor_scalar(out=sg_t[:], in0=sg_t[:], scalar1=0.5,
                                    scalar2=3.0, op0=Alu.add, op1=Alu.min)
            s2 = sc.tile([P, GPC], F32, tag="s2")
            nc.vector.tensor_tensor(out=s2[:], in0=sg_t[:], in1=sg_t[:],
                                    op=Alu.mult)
            nc.vector.tensor_scalar(out=s2[:], in0=s2[:], scalar1=4.0,
                                    scalar2=1e-8, op0=Alu.mult, op1=Alu.add)
            rs = sc.tile([P, GPC], F32, tag="rs")
            nc.vector.reciprocal(out=rs[:], in_=s2[:])

            # tanh(res_raw) via exp: 1 - 2/(exp(2x)+1)
            th = sc.tile([P, GK], F32, tag="th")
            nc.scalar.activation(out=th[:], in_=o3[:, :, 2:7], func=Act.Exp,
                                 scale=2.0)
            nc.vector.tensor_scalar(out=th[:], in0=th[:], scalar1=1.0,
                                    scalar2=None, op0=Alu.add)
            nc.vector.reciprocal(out=th[:], in_=th[:])
            nc.vector.tensor_scalar(out=th[:], in0=th[:], scalar1=-2.0,
                                    scalar2=1.0, op0=Alu.mult, op1=Alu.add)
            # sigmoid(gate_raw) via exp: 1/(exp(-x)+1)
            gt = sc.tile([P, GK], F32, tag="gt")
            nc.scalar.activation(out=gt[:], in_=o3[:, :, 7:12], func=Act.Exp,
                                 scale=-1.0)
            nc.vector.tensor_scalar(out=gt[:], in0=gt[:], scalar1=1.0,
                                    scalar2=None, op0=Alu.add)
            nc.vector.reciprocal(out=gt[:], in_=gt[:])

            off = sc.tile([P, GK], F32, tag="off")
            nc.vector.tensor_tensor(out=off[:], in0=_bc(r_t[:], K),
                                    in1=_bc_mid(base_sb[:], GPC), op=Alu.mult)
            nc.vector.scalar_tensor_tensor(out=off[:], in0=th[:], scalar=0.5,
                                           in1=off[:], op0=Alu.mult,
                                           op1=Alu.add)
            dix = sc.tile([P, GK], F32, tag="dix")
            nc.vector.scalar_tensor_tensor(
                out=dix[:], in0=off[:], scalar=float(DXSCALE),
                in1=_bc(xq[:, g0:g0 + GPC], K), op0=Alu.mult, op1=Alu.add)
            nc.vector.tensor_scalar(out=dix[:], in0=dix[:], scalar1=1.0,
                                    scalar2=0.5, op0=Alu.add, op1=Alu.mult)
            nc.vector.tensor_scalar(out=dix[:], in0=dix[:],
                                    scalar1=float(IXSCALE), scalar2=0.0,
                                    op0=Alu.mult, op1=Alu.max)
            nc.vector.tensor_scalar(out=dix[:], in0=dix[:],
                                    scalar1=float(IXSCALE), scalar2=None,
                                    op0=Alu.min)
            fracd = sc.tile([P, GK], F32, tag="fracd")
            i0fd = sc.tile([P, GK], F32, tag="i0fd")
            ti_d = sc.tile([P, GK], I32, tag="tid")
            nc.vector.tensor_copy(out=ti_d[:], in_=dix[:])
            nc.vector.tensor_copy(out=i0fd[:], in_=ti_d[:])
            gt_d = sc.tile([P, GK], F32, tag="gtd")
            nc.vector.tensor_tensor(out=gt_d[:], in0=i0fd[:], in1=dix[:],
                                    op=Alu.is_gt)
            nc.vector.tensor_tensor(out=i0fd[:], in0=i0fd[:], in1=gt_d[:],
                                    op=Alu.subtract)
            nc.vector.tensor_scalar(out=i0fd[:], in0=i0fd[:],
                                    scalar1=float(L - 2), scalar2=None,
                                    op0=Alu.min)
            nc.vector.tensor_tensor(out=fracd[:], in0=dix[:], in1=i0fd[:],
                                    op=Alu.subtract)

            o2 = sc.tile([P, GK], F32, tag="o2")
            nc.vector.tensor_tensor(out=o2[:], in0=off[:], in1=off[:],
                                    op=Alu.mult)
            nc.vector.tensor_tensor(out=o2[:], in0=o2[:], in1=_bc(rs[:], K),
                                    op=Alu.mult)
            w_t = sc.tile([P, GK], F32, tag="wt")
            nc.scalar.activation(out=w_t[:], in_=o2[:], func=Act.Exp,
                                 scale=-0.5)
            nc.vector.tensor_tensor(out=w_t[:], in0=w_t[:], in1=gt[:],
                                    op=Alu.mult)
            wsum = sc.tile([P, GPC], F32, tag="wsum")
            w_v = w_t[:].rearrange("p (g k) -> p g k", k=K)
            nc.vector.tensor_reduce(out=wsum[:], in_=w_v,
                                    axis=mybir.AxisListType.X, op=Alu.add)
            nc.vector.tensor_scalar(out=wsum[:], in0=wsum[:], scalar1=1e-8,
                                    scalar2=None, op0=Alu.add)
            rn = sc.tile([P, GPC], F32, tag="rn")
            nc.vector.reciprocal(out=rn[:], in_=wsum[:])
            wn = sc.tile([P, GK], F32, tag="wn")
            nc.vector.tensor_tensor(out=wn[:], in0=w_t[:], in1=_bc(rn[:], K),
                                    op=Alu.mult)
            c1 = sc.tile([P, GK], F32, tag="c1")
            nc.vector.tensor_tensor(out=c1[:], in0=wn[:], in1=fracd[:],
                                    op=Alu.mult)
            c0 = sc.tile([P, GK], F32, tag="c0")
            nc.vector.tensor_tensor(out=c0[:], in0=wn[:], in1=c1[:],
                                    op=Alu.subtract)

            if INDIRECT:
                # deform offset pairs [P, GK, 2] i32 = [i0; i0+1]
                didx = sc.tile([P, GK, 2], I32, tag="didx")
                i0p1 = sc.tile([P, GK], F32, tag="i0p1")
                nc.vector.tensor_scalar(out=i0p1[:], in0=i0fd[:], scalar1=1.0,
                                        scalar2=None, op0=Alu.add)
                nc.vector.tensor_copy(out=didx[:, :, 0], in_=i0fd[:])
                nc.vector.tensor_copy(out=didx[:, :, 1], in_=i0p1[:])

                # one indirect gather for all 5 taps' row-pairs of this chunk
                Gd = gathd.tile([P, GK, 2 * C], BF16, tag="gd")
                nc.gpsimd.indirect_dma_start(
                    out=Gd[:].rearrange("p n (r c) -> p (n r) c", r=2),
                    out_offset=None,
                    in_=featT.ap(),
                    in_offset=IndirectOffsetOnAxis(ap=didx[:], axis=0))

                def gsl(k, gi, half):
                    n = gi * K + k
                    return Gd[:, n, half * C:(half + 1) * C]
            else:
                wrepD = wdp.tile([P, K, NI // 16], I16, tag="wd")
                wrapped_idx(i0fd[:], K, wrepD)
                Gds = []
                for k in range(K):
                    Gd = gathd.tile([P, GPC, 2 * C], BF16, tag="gd")
                    nc.gpsimd.dma_gather(
                        out_ap=Gd[:], in_ap=gsrc,
                        idxs_ap=wrepD[:, k, :], num_idxs=NI, num_idxs_reg=NI,
                        elem_size=2 * C, elem_step=C, queue_num=next_q())
                    Gds.append(Gd)

                def gsl(k, gi, half):
                    return Gds[k][:, gi, half * C:(half + 1) * C]
            return gsl, c0, c1

        def combine_and_store(gsl, c0, c1, ch):
            accV = accp.tile([P, GPC, C], BF16 if ACT_OFFLOAD else F32,
                             tag="accV")
            for gi in range(GPC):
                acc = accV[:, gi, :]
                if ACT_OFFLOAD and gi < ACT_GIS:
                    # ACT: prod = row * c (per-partition scale); DVE: bf16
                    # tensor_tensor adds (2x mode) reduce the 10 products.
                    prods = []
                    for k in range(K):
                        n = gi * K + k
                        for half, cw in ((0, c0), (1, c1)):
                            pr = prodp.tile([P, C], BF16, tag="pr")
                            nc.scalar.activation(
                                out=pr[:], in_=gsl(k, gi, half), func=Act.Copy,
                                scale=cw[:, n:n + 1])
                            prods.append(pr)
                    nc.vector.tensor_tensor(out=acc, in0=prods[0][:],
                                            in1=prods[1][:], op=Alu.add)
                    for pr in prods[2:]:
                        nc.vector.tensor_tensor(out=acc, in0=acc, in1=pr[:],
                                                op=Alu.add)
                    continue
                for k in range(K):
                    n = gi * K + k
                    csc0 = c0[:, n:n + 1]
                    csc1 = c1[:, n:n + 1]
                    if k == 0:
                        nc.vector.tensor_scalar(out=acc, in0=gsl(k, gi, 0),
                                                scalar1=csc0, scalar2=None,
                                                op0=Alu.mult)
                    else:
                        nc.vector.scalar_tensor_tensor(
                            out=acc, in0=gsl(k, gi, 0), scalar=csc0, in1=acc,
                            op0=Alu.mult, op1=Alu.add)
                    nc.vector.scalar_tensor_tensor(
                        out=acc, in0=gsl(k, gi, 1), scalar=csc1, in1=acc,
                        op0=Alu.mult, op1=Alu.add)
            outv = out.ap().rearrange("(g p) c -> p g c", p=P)
            g0 = ch * GPC
            nc.sync.dma_start(out=outv[:, g0:g0 + GPC, :], in_=accV[:])

        # Emit fe(last) right after fe(last-1), before combine(last-2): the
        # last chunk's index chain then runs ahead of the combine products in
        # the ACT/DVE queues instead of stalling its gathers behind them.
        args = []
        for ch in range(NCH):
            args.append(front_end(ch))
            if 1 <= ch < NCH - 2:
                combine_and_store(*args[ch - 1], ch - 1)
        for ch in range(max(NCH - 3, 0), NCH):
            combine_and_store(*args[ch], ch)


_PROGRAM = None


def _get_program():
    global _PROGRAM
    if _PROGRAM is None:
        _PROGRAM = build_program()
    return _PROGRAM


def make_in_maps(feat_1d, coords_1d, cell_1d, W1, b1, Wr, br, W3, b3):
    """Build the 8 per-core input dicts from full inputs."""
    from ml_dtypes import bfloat16
    f32 = np.float32
    W1 = np.asarray(W1, f32)
    wr1 = (np.asarray(Wr, f32) + np.eye(H, dtype=f32)).astype(bfloat16)
    base = np.array([-2.0, -1.0, 0.0, 1.0, 2.0], f32)
    base128 = np.broadcast_to(base, (P, K)).copy()
    b3rep = np.broadcast_to(np.asarray(b3, f32), (P, 12)).copy()
    sel = np.zeros((P, 8, 128), f32)
    for a in range(8):
        for m in range(128):
            sel[16 * a + m % 16, a, m] = 1.0
    shared = {
        "sel8": sel.reshape(P, 8 * 128),
        "w1a0": np.ascontiguousarray(W1[0:128]).astype(bfloat16),
        "w1a1": np.ascontiguousarray(W1[128:256]).astype(bfloat16),
        "wxc": np.ascontiguousarray(W1[256:258]).astype(bfloat16),
        "b1c": np.asarray(b1, f32).reshape(H, 1).copy(),
        "wr1": wr1,
        "brc": np.asarray(br, f32).reshape(H, 1).copy(),
        "w3c": np.asarray(W3, f32).astype(bfloat16),
        "b3rep": b3rep,
        "base128": base128,
    }
    featTs = [np.ascontiguousarray(np.asarray(feat_1d[b], f32).T)
              .astype(bfloat16) for b in range(B)]
    in_maps = []
    for core in range(NCORES):
        b = core // 2
        s = core % 2
        sl = slice(s * Q, (s + 1) * Q)
        cds = np.ascontiguousarray(np.asarray(coords_1d[b, sl, 0], f32))
        cel = np.ascontiguousarray(np.asarray(cell_1d[b, sl, 0], f32))
        xcb = np.stack([cds, cel]).astype(bfloat16)
        in_maps.append({
            "featT": featTs[b],
            "coords": cds,
            "xcb": xcb,
            **shared,
        })
    return in_maps


def kernel(feat_1d, coords_1d, cell_1d, W1, b1, Wr, br, W3, b3):
    from concourse.bass_utils import run_bass_kernel_spmd
    nc = _get_program()
    in_maps = make_in_maps(feat_1d, coords_1d, cell_1d, W1, b1, Wr, br, W3, b3)
    res = run_bass_kernel_spmd(nc, in_maps, core_ids=list(range(NCORES)))
    outf = np.zeros((B, N, C), np.float32)
    for core in range(NCORES):
        b = core // 2
        s = core % 2
        outf[b, s * Q:(s + 1) * Q, :] = np.asarray(
            res.results[core]["out"]).astype(np.float32)
    return outf


# revision 66
# speedup vs baseline: 1.1106x; 1.0200x over previous
"""Trainium2 Bass kernel for nn_DeformableDynamicGather1D.

Sharding: 8 cores = 4 batches x 2 query-halves. Each core handles one batch's
feat and Q=4096 queries.

Host prep: feat is transposed to feat_T [L, C] and cast to bf16 on the host
(layout/precision prep, same class as the existing weight repacking). Router
weights are pre-cast to bf16; Wr has identity folded in; b3 is replicated to
[128, 12]; coords/cell are pre-cast to a bf16 [2, Q] tile for the MLP tail.

Device pipeline, per 1024-query chunk (4 chunks, software-pipelined so chunk
N's gathers run while chunk N-1 combines):

  1. Anchor: bilinear indices from coords (f32 DVE math, explicit [i0; i0+1]
     int32 offset pairs); indirect_dma_start row gather from feat_T (hardware
     dynamic DGE -- the DMA engine reads the offset table, Pool is not
     blocked); lerp on DVE (bf16); PE-transpose into channel-major rin.
  2. MLP on PE in bf16 (1 cycle/row): h = lrelu(rin@W1+b1) via ACT Prelu;
     g = lrelu(h@(Wr+I)+br); out3 = g@W3 + b3 (b3 added on DVE).
  3. Scalar stage: softplus via Abs/Exp/Ln, tanh/sigmoid via Exp + DVE
     reciprocal -- every ACT func lives in the natural_log_exp table, so no
     act-table reloads. Produces deform offset pairs and weights c0/c1.
  4. Deform: one indirect gather per chunk fetches all 5 taps' row-pairs
     (bf16); combine with scalar_tensor_tensor FMAs on DVE into a f32
     accumulator; per-chunk output DMA overlaps the next chunk's compute.

Query <-> tile coordinates: q = g*128 + p (tile [128 p, G g]).
"""
import os
import sys

for _p in ("/opt/trn_rl_repo", "/root/.axon_site/_ro/trn_rl_repo"):
    if os.path.isdir(_p) and _p not in sys.path:
        sys.path.append(_p)

import numpy as np
import concourse.bass as bass
import concourse.bacc as bacc
import concourse.tile as tile
from concourse import mybir
from concourse.bass import AP, IndirectOffsetOnAxis
from concourse.masks import make_identity

F32 = mybir.dt.float32
BF16 = mybir.dt.bfloat16
I32 = mybir.dt.int32
Act = mybir.ActivationFunctionType
Alu = mybir.AluOpType

P = 128          # partitions
G = 32           # q = g*128 + p
Q = P * G        # 4096 queries per core
C = 256          # channels
L = 4096         # feat length
H = 64           # hidden
K = 5            # taps
NCORES = 8
B, N = 4, 8192   # full problem
NI = int(os.environ.get("KERNEL_NI", "1024"))  # queries per chunk
NCH = Q // NI    # chunks
GPC = NI // P    # g-columns per chunk
GK = GPC * K

IXSCALE = np.float32(float(L - 1))          # 4095
DXSCALE = np.float32(2.0 / max(L - 1, 1))   # reference scale_x

# CoreSim lacks Prelu: sim-safe mode uses Identity + DVE leaky instead
# (numerically identical; only used by the local debug harness).
SIM_SAFE = os.environ.get("KERNEL_SIM_SAFE") == "1"

# Gather engine: per-tap blocking dma_gather with the wrapped-i16 idx layout
# (default), vs indirect_dma_start (hardware dynamic DGE -- passes CoreSim
# but aborts at runtime on this hardware stack; kept for reference).
INDIRECT = os.environ.get("KERNEL_GATHER", "dma_gather") == "indirect"


def _bc(ap2d: AP, extra: int) -> AP:
    """Broadcast a [p, n] AP to [p, n, extra] with stride-0 inner dim."""
    return AP(tensor=ap2d.tensor, offset=ap2d.offset,
              ap=[*ap2d.ap, [0, extra]])


def _bc_mid(ap2d: AP, mid: int) -> AP:
    """Broadcast a [p, n] AP to [p, mid, n] with stride-0 middle dim."""
    return AP(tensor=ap2d.tensor, offset=ap2d.offset,
              ap=[ap2d.ap[0], [0, mid], ap2d.ap[1]])


# Multi-queue SWDGE is unsupported under Tile: DMASW lane sems get locked to
# the first queue that uses them and the scheduler's lane rotation cannot be
# aligned with a per-call queue rotation (CoreSim flags the conflict).
NQUEUES = int(os.environ.get("KERNEL_NQ", "1"))

# Offload most combine FMAs to the Scalar engine: ACT computes the
# per-partition-scaled products (out = gathered_row * c), DVE reduces them
# with bf16 tensor_tensor adds (2x mode) into bf16 accumulators; output
# tensor becomes bf16 and the host converts to f32. g-columns >= ACT_GIS
# stay on the DVE scalar_tensor_tensor path.
ACT_OFFLOAD = os.environ.get("KERNEL_ACTOFF", "1") == "1"
ACT_GIS = 6


def build_program():
    nc = bacc.Bacc("TRN2", target_bir_lowering=False, debug=False,
                   num_devices=NCORES, num_swdge_queues=NQUEUES)

    featT = nc.dram_tensor("featT", [L, C], BF16, kind="ExternalInput")
    coords = nc.dram_tensor("coords", [Q], F32, kind="ExternalInput")
    xcb = nc.dram_tensor("xcb", [2, Q], BF16, kind="ExternalInput")
    w1a0 = nc.dram_tensor("w1a0", [128, H], BF16, kind="ExternalInput")
    w1a1 = nc.dram_tensor("w1a1", [128, H], BF16, kind="ExternalInput")
    wxc = nc.dram_tensor("wxc", [2, H], BF16, kind="ExternalInput")
    b1c = nc.dram_tensor("b1c", [H, 1], F32, kind="ExternalInput")
    wr1 = nc.dram_tensor("wr1", [H, H], BF16, kind="ExternalInput")
    brc = nc.dram_tensor("brc", [H, 1], F32, kind="ExternalInput")
    w3c = nc.dram_tensor("w3c", [H, 12], BF16, kind="ExternalInput")
    b3rep = nc.dram_tensor("b3rep", [P, 12], F32, kind="ExternalInput")
    base128 = nc.dram_tensor("base128", [P, K], F32, kind="ExternalInput")
    sel8 = nc.dram_tensor("sel8", [P, 8 * 128], F32, kind="ExternalInput")
    out = nc.dram_tensor("out", [Q, C], BF16 if ACT_OFFLOAD else F32,
                         kind="ExternalOutput")

    with tile.TileContext(nc) as tc:
        _body(nc, tc, featT, coords, xcb, w1a0, w1a1, wxc, b1c, wr1, brc,
              w3c, b3rep, base128, sel8, out)
    nc.compile()
    return nc


def _body(nc, tc, featT, coords, xcb, w1a0, w1a1, wxc, b1c, wr1, brc,
          w3c, b3rep, base128, sel8, out):
    import contextlib
    ctx = contextlib.ExitStack()
    with ctx:
        big = NI > 1024   # larger chunks: shrink buffer rings to fit SBUF
        const = ctx.enter_context(tc.tile_pool(name="const", bufs=1))
        rpool = ctx.enter_context(tc.tile_pool(name="rpool", bufs=1 if big else 2))
        gatha = ctx.enter_context(tc.tile_pool(name="gatha", bufs=1 if big else 4))
        gathd = ctx.enter_context(
            tc.tile_pool(name="gathd", bufs=2 if INDIRECT else (6 if big else 14)))
        fab = ctx.enter_context(tc.tile_pool(name="fab", bufs=1 if big else 2))
        sc = ctx.enter_context(tc.tile_pool(name="scal", bufs=3))
        wdp = ctx.enter_context(tc.tile_pool(name="wdp", bufs=2))
        accp = ctx.enter_context(tc.tile_pool(name="accp", bufs=1 if big else 2))
        prodp = ctx.enter_context(tc.tile_pool(name="prodp", bufs=12))
        tps = ctx.enter_context(tc.tile_pool(name="tps", bufs=2, space="PSUM"))
        mmps = ctx.enter_context(tc.tile_pool(name="mmps", bufs=2, space="PSUM"))
        l3ps = ctx.enter_context(tc.tile_pool(name="l3ps", bufs=2, space="PSUM"))

        ident = const.tile([P, P], BF16)
        make_identity(nc, ident[:])

        # weights / constants
        w1a0_sb = const.tile([128, H], BF16)
        w1a1_sb = const.tile([128, H], BF16)
        wxc_sb = const.tile([2, H], BF16)
        b1_sb = const.tile([H, 1], F32)
        wr1_sb = const.tile([H, H], BF16)
        br_sb = const.tile([H, 1], F32)
        w3_sb = const.tile([H, 12], BF16)
        b3_sb = const.tile([P, 12], F32)
        base_sb = const.tile([P, K], F32)
        xcb_sb = const.tile([2, Q], BF16)
        loads = [(w1a0_sb, w1a0), (w1a1_sb, w1a1), (wxc_sb, wxc),
                 (b1_sb, b1c), (wr1_sb, wr1), (br_sb, brc),
                 (w3_sb, w3c), (b3_sb, b3rep), (base_sb, base128),
                 (xcb_sb, xcb)]
        if not INDIRECT:
            sel_sb = const.tile([P, 8 * 128], F32)
            loads.append((sel_sb, sel8))
        for dst, src in loads:
            nc.sync.dma_start(out=dst[:], in_=src.ap())

        # feat_T row-pair view for dma_gather: idx i -> elems [256*i, +512)
        gsrc = AP(tensor=featT.ap().tensor, offset=0,
                  ap=[[C, L - 1], [1, 2 * C]])
        I16 = mybir.dt.int16
        qctr = [0]

        def next_q():
            q = qctr[0] % NQUEUES
            qctr[0] += 1
            return q

        def wrapped_idx(vf32_ap, nk, wrep):
            """Build replicated wrapped int16 idx tile from a query-major f32
            index tile V [128, nk*Gq] ((g, k)-major cols: n = g*nk + k) via 8
            selection matmuls; wrep is [128, nk, (128*Gq)//16] i16."""
            Gq = vf32_ap.shape[-1] // nk
            for a in range(8):
                psw = l3ps.tile([P, GK], F32, tag="psw", space="PSUM")
                nc.tensor.matmul(
                    out=psw[:, 0:nk * Gq], lhsT=sel_sb[:, a * 128:(a + 1) * 128],
                    rhs=vf32_ap, start=True, stop=True)
                dst = AP(tensor=wrep[:].tensor, offset=wrep[:].offset + a,
                         ap=[wrep[:].ap[0], [8 * Gq, nk], [8, Gq]])
                src = AP(tensor=psw[:].tensor, offset=psw[:].offset,
                         ap=[psw[:].ap[0], [1, nk], [nk, Gq]])
                nc.vector.tensor_copy(out=dst, in_=src)

        # ---- anchor index math, full Q upfront (query-major [P, G]) ----
        xq = const.tile([P, G], F32)
        nc.sync.dma_start(
            out=xq[:],
            in_=AP(tensor=coords.ap().tensor, offset=0, ap=[[1, P], [P, G]]))
        ixf = const.tile([P, G], F32)
        nc.vector.tensor_scalar(out=ixf[:], in0=xq[:], scalar1=1.0,
                                scalar2=0.5, op0=Alu.add, op1=Alu.mult)
        nc.vector.tensor_scalar(out=ixf[:], in0=ixf[:], scalar1=float(IXSCALE),
                                scalar2=0.0, op0=Alu.mult, op1=Alu.max)
        nc.vector.tensor_scalar(out=ixf[:], in0=ixf[:], scalar1=float(IXSCALE),
                                scalar2=None, op0=Alu.min)
        # i0 = min(floor(ix), L-2); frac = ix - i0 (floor via int convert +
        # fixup, correct for both trunc and round-nearest convert modes)
        fraca = const.tile([P, G], F32)
        i0fa = const.tile([P, G], F32)
        ti_a = const.tile([P, G], I32)
        nc.vector.tensor_copy(out=ti_a[:], in_=ixf[:])
        nc.vector.tensor_copy(out=i0fa[:], in_=ti_a[:])
        gt_a = const.tile([P, G], F32)
        nc.vector.tensor_tensor(out=gt_a[:], in0=i0fa[:], in1=ixf[:],
                                op=Alu.is_gt)
        nc.vector.tensor_tensor(out=i0fa[:], in0=i0fa[:], in1=gt_a[:],
                                op=Alu.subtract)
        nc.vector.tensor_scalar(out=i0fa[:], in0=i0fa[:], scalar1=float(L - 2),
                                scalar2=None, op0=Alu.min)
        nc.vector.tensor_tensor(out=fraca[:], in0=ixf[:], in1=i0fa[:],
                                op=Alu.subtract)
        if INDIRECT:
            # anchor offset pairs [P, G, 2] i32 = [i0; i0+1]
            aidx = const.tile([P, G, 2], I32)
            i0p1a = const.tile([P, G], F32)
            nc.vector.tensor_scalar(out=i0p1a[:], in0=i0fa[:], scalar1=1.0,
                                    scalar2=None, op0=Alu.add)
            nc.vector.tensor_copy(out=aidx[:, :, 0], in_=i0fa[:])
            nc.vector.tensor_copy(out=aidx[:, :, 1], in_=i0p1a[:])
        else:
            wrapA = const.tile([P, 1, Q // 16], I16)
            wrapped_idx(i0fa[:], 1, wrapA)

        # All anchor gathers issued upfront: each chunk's MLP/scalar chain
        # (which gates its deform gathers) then overlaps the previous chunk's
        # deform burst instead of queueing its anchor behind it on Pool.
        Gas = []
        for ch in range(NCH):
            Ga = gatha.tile([P, GPC, 2 * C], BF16, tag="ga")
            if INDIRECT:
                nc.gpsimd.indirect_dma_start(
                    out=Ga[:].rearrange("p g (r c) -> p (g r) c", r=2),
                    out_offset=None,
                    in_=featT.ap(),
                    in_offset=IndirectOffsetOnAxis(
                        ap=aidx[:, ch * GPC:(ch + 1) * GPC, :], axis=0))
            else:
                for s in range(NI // 1024):
                    f0 = ch * (NI // 16) + s * 64
                    nc.gpsimd.dma_gather(
                        out_ap=Ga[:, s * 8:(s + 1) * 8, :], in_ap=gsrc,
                        idxs_ap=wrapA[:, 0, f0:f0 + 64],
                        num_idxs=1024, num_idxs_reg=1024, elem_size=2 * C,
                        elem_step=C, queue_num=next_q())
            Gas.append(Ga)

        # ---------------- software-pipelined chunk loop ----------------
        # fe(ch) (which issues ch's deform gathers) is emitted before
        # combine(ch-1) so gathers stay a chunk ahead of the combines.
        pend = [None]

        def front_end(ch):
            g0 = ch * GPC
            Ga = Gas[ch]

            # lerp: d = f1 - f0 (bf16 2x); fa = frac*d + f0 per g-column
            d = fab.tile([P, GPC, C], BF16, tag="dl")
            nc.vector.tensor_tensor(out=d[:], in0=Ga[:, :, C:2 * C],
                                    in1=Ga[:, :, 0:C], op=Alu.subtract)
            rin0 = rpool.tile([P, NI], BF16, tag="rin0")
            rin1 = rpool.tile([P, NI], BF16, tag="rin1")
            for gi in range(GPC):
                g = g0 + gi
                fa = fab.tile([P, C], BF16, tag="fa")
                nc.vector.scalar_tensor_tensor(
                    out=fa[:], in0=d[:, gi, :], scalar=fraca[:, g:g + 1],
                    in1=Ga[:, gi, 0:C], op0=Alu.mult, op1=Alu.add)
                for hh in range(2):
                    tp = tps.tile([P, P], BF16, tag="tp", space="PSUM")
                    nc.tensor.transpose(out=tp[:],
                                        in_=fa[:, hh * 128:(hh + 1) * 128],
                                        identity=ident[:])
                    rdst = (rin0 if hh == 0 else rin1)
                    nc.scalar.copy(out=rdst[:, gi * 128:(gi + 1) * 128],
                                   in_=tp[:])

            # MLP (bf16): h = lrelu(rin@W1 + b1); g = lrelu(h + h@Wr + br)
            hb = rpool.tile([H, NI], BF16, tag="hb")
            gg = rpool.tile([H, NI], BF16, tag="gg")
            for n in range(NI // 512):
                sl = slice(n * 512, (n + 1) * 512)
                gsl = slice(ch * NI + n * 512, ch * NI + (n + 1) * 512)
                ps1 = mmps.tile([H, 512], F32, tag="ps1", space="PSUM")
                nc.tensor.matmul(out=ps1[:], lhsT=w1a0_sb[:], rhs=rin0[:, sl],
                                 start=True, stop=False)
                nc.tensor.matmul(out=ps1[:], lhsT=w1a1_sb[:], rhs=rin1[:, sl],
                                 start=False, stop=False)
                nc.tensor.matmul(out=ps1[:], lhsT=wxc_sb[:], rhs=xcb_sb[:, gsl],
                                 start=False, stop=True)
                def lrelu(dst, ps, bias_sb, tag):
                    if not SIM_SAFE:
                        nc.scalar.activation(out=dst, in_=ps, func=Act.Prelu,
                                             bias=bias_sb[:, :], scale=1.0,
                                             alpha=0.2)
                    else:
                        t = sc.tile([H, 512], F32, tag=tag)
                        nc.scalar.activation(out=t[:], in_=ps,
                                             func=Act.Identity,
                                             bias=bias_sb[:, :], scale=1.0)
                        nc.vector.scalar_tensor_tensor(
                            out=dst, in0=t[:], scalar=0.2, in1=t[:],
                            op0=Alu.mult, op1=Alu.max)

                lrelu(hb[:, sl], ps1[:], b1_sb, "lr1")
                ps2 = mmps.tile([H, 512], F32, tag="ps1", space="PSUM")
                nc.tensor.matmul(out=ps2[:], lhsT=wr1_sb[:], rhs=hb[:, sl],
                                 start=True, stop=True)
                lrelu(gg[:, sl], ps2[:], br_sb, "lr2")

            # out3 = g@W3 (+ b3 on DVE), query-major [P, GPC, 12]
            o3 = rpool.tile([P, GPC, 12], F32, tag="o3")
            for gi in range(GPC):
                ps3 = l3ps.tile([P, 12], F32, tag="ps3", space="PSUM")
                nc.tensor.matmul(out=ps3[:],
                                 lhsT=gg[:, gi * 128:(gi + 1) * 128],
                                 rhs=w3_sb[:], start=True, stop=True)
                nc.scalar.copy(out=o3[:, gi, :], in_=ps3[:])
            nc.vector.tensor_tensor(out=o3[:], in0=o3[:],
                                    in1=_bc_mid(b3_sb[:], GPC), op=Alu.add)

            # ---- scalar stage (tiles [P, GPC] / [P, GK] f32) ----
            def softplus(dst, src_ap, tag):
                a = sc.tile([P, GPC], F32, tag=tag + "a")
                nc.scalar.activation(out=a[:], in_=src_ap, func=Act.Abs)
                e = sc.tile([P, GPC], F32, tag=tag + "e")
                nc.scalar.activation(out=e[:], in_=a[:], func=Act.Exp,
                                     scale=-1.0)
                lg = sc.tile([P, GPC], F32, tag=tag + "l")
                nc.scalar.activation(out=lg[:], in_=e[:], func=Act.Ln,
                                     bias=1.0, scale=1.0)
                m = sc.tile([P, GPC], F32, tag=tag + "m")
                nc.vector.tensor_scalar(out=m[:], in0=src_ap, scalar1=0.0,
                                        scalar2=None, op0=Alu.max)
                nc.vector.tensor_tensor(out=dst, in0=lg[:], in1=m[:],
                                        op=Alu.add)

            r_t = sc.tile([P, GPC], F32, tag="rt")
            softplus(r_t[:], o3[:, :, 0], "spr")
            nc.vector.tensor_scalar(out=r_t[:], in0=r_t[:], scalar1=0.3,
                                    scalar2=2.0, op0=Alu.add, op1=Alu.min)
            sg_t = sc.tile([P, GPC], F32, tag="sgt")
            softplus(sg_t[:], o3[:, :, 1], "sps")
            nc.vector.tensor_scalar(out=sg_t[:], in0=sg_t[:], scalar1=0.5,
                                    scalar2=3.0, op0=Alu.add, op1=Alu.min)
            s2 = sc.tile([P, GPC], F32, tag="s2")
            nc.vector.tensor_tensor(out=s2[:], in0=sg_t[:], in1=sg_t[:],
                                    op=Alu.mult)
            nc.vector.tensor_scalar(out=s2[:], in0=s2[:], scalar1=4.0,
                                    scalar2=1e-8, op0=Alu.mult, op1=Alu.add)
            rs = sc.tile([P, GPC], F32, tag="rs")
            nc.vector.reciprocal(out=rs[:], in_=s2[:])

            # tanh(res_raw) via exp: 1 - 2/(exp(2x)+1)
            th = sc.tile([P, GK], F32, tag="th")
            nc.scalar.activation(out=th[:], in_=o3[:, :, 2:7], func=Act.Exp,
                                 scale=2.0)
            nc.vector.tensor_scalar(out=th[:], in0=th[:], scalar1=1.0,
                                    scalar2=None, op0=Alu.add)
            nc.vector.reciprocal(out=th[:], in_=th[:])
            nc.vector.tensor_scalar(out=th[:], in0=th[:], scalar1=-2.0,
                                    scalar2=1.0, op0=Alu.mult, op1=Alu.add)
            # sigmoid(gate_raw) via exp: 1/(exp(-x)+1)
            gt = sc.tile([P, GK], F32, tag="gt")
            nc.scalar.activation(out=gt[:], in_=o3[:, :, 7:12], func=Act.Exp,
                                 scale=-1.0)
            nc.vector.tensor_scalar(out=gt[:], in0=gt[:], scalar1=1.0,
                                    scalar2=None, op0=Alu.add)
            nc.vector.reciprocal(out=gt[:], in_=gt[:])

            off = sc.tile([P, GK], F32, tag="off")
            nc.vector.tensor_tensor(out=off[:], in0=_bc(r_t[:], K),
                                    in1=_bc_mid(base_sb[:], GPC), op=Alu.mult)
            nc.vector.scalar_tensor_tensor(out=off[:], in0=th[:], scalar=0.5,
                                           in1=off[:], op0=Alu.mult,
                                           op1=Alu.add)
            dix = sc.tile([P, GK], F32, tag="dix")
            nc.vector.scalar_tensor_tensor(
                out=dix[:], in0=off[:], scalar=float(DXSCALE),
                in1=_bc(xq[:, g0:g0 + GPC], K), op0=Alu.mult, op1=Alu.add)
            nc.vector.tensor_scalar(out=dix[:], in0=dix[:], scalar1=1.0,
                                    scalar2=0.5, op0=Alu.add, op1=Alu.mult)
            nc.vector.tensor_scalar(out=dix[:], in0=dix[:],
                                    scalar1=float(IXSCALE), scalar2=0.0,
                                    op0=Alu.mult, op1=Alu.max)
            nc.vector.tensor_scalar(out=dix[:], in0=dix[:],
                                    scalar1=float(IXSCALE), scalar2=None,
                                    op0=Alu.min)
            fracd = sc.tile([P, GK], F32, tag="fracd")
            i0fd = sc.tile([P, GK], F32, tag="i0fd")
            ti_d = sc.tile([P, GK], I32, tag="tid")
            nc.vector.tensor_copy(out=ti_d[:], in_=dix[:])
            nc.vector.tensor_copy(out=i0fd[:], in_=ti_d[:])
            gt_d = sc.tile([P, GK], F32, tag="gtd")
            nc.vector.tensor_tensor(out=gt_d[:], in0=i0fd[:], in1=dix[:],
                                    op=Alu.is_gt)
            nc.vector.tensor_tensor(out=i0fd[:], in0=i0fd[:], in1=gt_d[:],
                                    op=Alu.subtract)
            nc.vector.tensor_scalar(out=i0fd[:], in0=i0fd[:],
                                    scalar1=float(L - 2), scalar2=None,
                                    op0=Alu.min)
            nc.vector.tensor_tensor(out=fracd[:], in0=dix[:], in1=i0fd[:],
                                    op=Alu.subtract)

            o2 = sc.tile([P, GK], F32, tag="o2")
            nc.vector.tensor_tensor(out=o2[:], in0=off[:], in1=off[:],
                                    op=Alu.mult)
            nc.vector.tensor_tensor(out=o2[:], in0=o2[:], in1=_bc(rs[:], K),
                                    op=Alu.mult)
            w_t = sc.tile([P, GK], F32, tag="wt")
            nc.scalar.activation(out=w_t[:], in_=o2[:], func=Act.Exp,
                                 scale=-0.5)
            nc.vector.tensor_tensor(out=w_t[:], in0=w_t[:], in1=gt[:],
                                    op=Alu.mult)
            wsum = sc.tile([P, GPC], F32, tag="wsum")
            w_v = w_t[:].rearrange("p (g k) -> p g k", k=K)
            nc.vector.tensor_reduce(out=wsum[:], in_=w_v,
                                    axis=mybir.AxisListType.X, op=Alu.add)
            nc.vector.tensor_scalar(out=wsum[:], in0=wsum[:], scalar1=1e-8,
                                    scalar2=None, op0=Alu.add)
            rn = sc.tile([P, GPC], F32, tag="rn")
            nc.vector.reciprocal(out=rn[:], in_=wsum[:])
            wn = sc.tile([P, GK], F32, tag="wn")
            nc.vector.tensor_tensor(out=wn[:], in0=w_t[:], in1=_bc(rn[:], K),
                                    op=Alu.mult)
            c1 = sc.tile([P, GK], F32, tag="c1")
            nc.vector.tensor_tensor(out=c1[:], in0=wn[:], in1=fracd[:],
                                    op=Alu.mult)
            c0 = sc.tile([P, GK], F32, tag="c0")
            nc.vector.tensor_tensor(out=c0[:], in0=wn[:], in1=c1[:],
                                    op=Alu.subtract)

            if INDIRECT:
                # deform offset pairs [P, GK, 2] i32 = [i0; i0+1]
                didx = sc.tile([P, GK, 2], I32, tag="didx")
                i0p1 = sc.tile([P, GK], F32, tag="i0p1")
                nc.vector.tensor_scalar(out=i0p1[:], in0=i0fd[:], scalar1=1.0,
                                        scalar2=None, op0=Alu.add)
                nc.vector.tensor_copy(out=didx[:, :, 0], in_=i0fd[:])
                nc.vector.tensor_copy(out=didx[:, :, 1], in_=i0p1[:])

                # one indirect gather for all 5 taps' row-pairs of this chunk
                Gd = gathd.tile([P, GK, 2 * C], BF16, tag="gd")
                nc.gpsimd.indirect_dma_start(
                    out=Gd[:].rearrange("p n (r c) -> p (n r) c", r=2),
                    out_offset=None,
                    in_=featT.ap(),
                    in_offset=IndirectOffsetOnAxis(ap=didx[:], axis=0))

                def gsl(k, gi, half):
                    n = gi * K + k
                    return Gd[:, n, half * C:(half + 1) * C]
            else:
                wrepD = wdp.tile([P, K, NI // 16], I16, tag="wd")
                wrapped_idx(i0fd[:], K, wrepD)
                Gds = []
                for k in range(K):
                    Gd = gathd.tile([P, GPC, 2 * C], BF16, tag="gd")
                    nc.gpsimd.dma_gather(
                        out_ap=Gd[:], in_ap=gsrc,
                        idxs_ap=wrepD[:, k, :], num_idxs=NI, num_idxs_reg=NI,
                        elem_size=2 * C, elem_step=C, queue_num=next_q())
                    Gds.append(Gd)

                def gsl(k, gi, half):
                    return Gds[k][:, gi, half * C:(half + 1) * C]
            return gsl, c0, c1

        def combine_and_store(gsl, c0, c1, ch):
            accV = accp.tile([P, GPC, C], BF16 if ACT_OFFLOAD else F32,
                             tag="accV")
            for gi in range(GPC):
                acc = accV[:, gi, :]
                if ACT_OFFLOAD and gi < ACT_GIS:
                    # ACT: prod = row * c (per-partition scale); DVE: bf16
                    # tensor_tensor adds (2x mode) reduce the 10 products.
                    prods = []
                    for k in range(K):
                        n = gi * K + k
                        for half, cw in ((0, c0), (1, c1)):
                            pr = prodp.tile([P, C], BF16, tag="pr")
                            nc.scalar.activation(
                                out=pr[:], in_=gsl(k, gi, half), func=Act.Copy,
                                scale=cw[:, n:n + 1])
                            prods.append(pr)
                    nc.vector.tensor_tensor(out=acc, in0=prods[0][:],
                                            in1=prods[1][:], op=Alu.add)
                    for pr in prods[2:]:
                        nc.vector.tensor_tensor(out=acc, in0=acc, in1=pr[:],
                                                op=Alu.add)
                    continue
                for k in range(K):
                    n = gi * K + k
                    csc0 = c0[:, n:n + 1]
                    csc1 = c1[:, n:n + 1]
                    if k == 0:
                        nc.vector.tensor_scalar(out=acc, in0=gsl(k, gi, 0),
                                                scalar1=csc0, scalar2=None,
                                                op0=Alu.mult)
                    else:
                        nc.vector.scalar_tensor_tensor(
                            out=acc, in0=gsl(k, gi, 0), scalar=csc0, in1=acc,
                            op0=Alu.mult, op1=Alu.add)
                    nc.vector.scalar_tensor_tensor(
                        out=acc, in0=gsl(k, gi, 1), scalar=csc1, in1=acc,
                        op0=Alu.mult, op1=Alu.add)
            outv = out.ap().rearrange("(g p) c -> p g c", p=P)
            g0 = ch * GPC
            nc.sync.dma_start(out=outv[:, g0:g0 + GPC, :], in_=accV[:])

        # Emit fe(last) right after fe(last-1), before combine(last-2): the
        # last chunk's index chain then runs ahead of the combine products in
        # the ACT/DVE queues instead of stalling its gathers behind them.
        args = []
        for ch in range(NCH):
            args.append(front_end(ch))
            if 1 <= ch < NCH - 2:
                combine_and_store(*args[ch - 1], ch - 1)
        for ch in range(max(NCH - 3, 0), NCH):
            combine_and_store(*args[ch], ch)


_PROGRAM = None


def _get_program():
    global _PROGRAM
    if _PROGRAM is None:
        _PROGRAM = build_program()
    return _PROGRAM


def make_in_maps(feat_1d, coords_1d, cell_1d, W1, b1, Wr, br, W3, b3):
    """Build the 8 per-core input dicts from full inputs."""
    from ml_dtypes import bfloat16
    f32 = np.float32
    W1 = np.asarray(W1, f32)
    wr1 = (np.asarray(Wr, f32) + np.eye(H, dtype=f32)).astype(bfloat16)
    base = np.array([-2.0, -1.0, 0.0, 1.0, 2.0], f32)
    base128 = np.broadcast_to(base, (P, K)).copy()
    b3rep = np.broadcast_to(np.asarray(b3, f32), (P, 12)).copy()
    sel = np.zeros((P, 8, 128), f32)
    for a in range(8):
        for m in range(128):
            sel[16 * a + m % 16, a, m] = 1.0
    shared = {
        "sel8": sel.reshape(P, 8 * 128),
        "w1a0": np.ascontiguousarray(W1[0:128]).astype(bfloat16),
        "w1a1": np.ascontiguousarray(W1[128:256]).astype(bfloat16),
        "wxc": np.ascontiguousarray(W1[256:258]).astype(bfloat16),
        "b1c": np.asarray(b1, f32).reshape(H, 1).copy(),
        "wr1": wr1,
        "brc": np.asarray(br, f32).reshape(H, 1).copy(),
        "w3c": np.asarray(W3, f32).astype(bfloat16),
        "b3rep": b3rep,
        "base128": base128,
    }
    featTs = [np.ascontiguousarray(np.asarray(feat_1d[b], f32).T)
              .astype(bfloat16) for b in range(B)]
    in_maps = []
    for core in range(NCORES):
        b = core // 2
        s = core % 2
        sl = slice(s * Q, (s + 1) * Q)
        cds = np.ascontiguousarray(np.asarray(coords_1d[b, sl, 0], f32))
        cel = np.ascontiguousarray(np.asarray(cell_1d[b, sl, 0], f32))
        xcb = np.stack([cds, cel]).astype(bfloat16)
        in_maps.append({
            "featT": featTs[b],
            "coords": cds,
            "xcb": xcb,
            **shared,
        })
    return in_maps


def kernel(feat_1d, coords_1d, cell_1d, W1, b1, Wr, br, W3, b3):
    from concourse.bass_utils import run_bass_kernel_spmd
    nc = _get_program()
    in_maps = make_in_maps(feat_1d, coords_1d, cell_1d, W1, b1, Wr, br, W3, b3)
    res = run_bass_kernel_spmd(nc, in_maps, core_ids=list(range(NCORES)))
    outf = np.zeros((B, N, C), np.float32)
    for core in range(NCORES):
        b = core // 2
        s = core % 2
        outf[b, s * Q:(s + 1) * Q, :] = np.asarray(
            res.results[core]["out"]).astype(np.float32)
    return outf
